# revision 20
# baseline (speedup 1.0000x reference)
"""MHCLiteBlock Trainium2 kernel.

Data-parallel over T across 8 NeuronCores (1024 tokens/core); all params
replicated. Per core, per 128-token tile:

  1. SWDGE cast-DMA: x fp32 HBM -> xn bf16 SBUF (4 chunks).
  2. ACT Square+accum on xn -> ssq; irms = exp(-0.5*ln(ssq/NC + eps))
     (ln/exp keep every ACT transcendental in ONE table set).
  3. DMA-xbar-transpose xn -> xT chunks [128c, 128t].
  4. proj (PE): proj[t, 32] = sum_k xT_k.T @ wallT_k directly in [t, .] layout.
  5. scaled = (proj * irms) * alpha + bias with alpha/bias negated on cols 0:8;
     eall = Exp(scaled): cols 0:8 = exp(-z) -> sigmoid via DVE 1/(1+u);
     cols 8:32 = softmax numerators. Soft permutation H via perm_aug matmul.
  6. li (DVE): libf = sum_m h_pre_m * x_m, bf16.
  7. M2 (PE): diff = liT.T @ (W_layer.T - I) + b_layer.
  8. Mixing (PE): out_n = sum_m diag(H[n,m]) @ x_m; DVE combine adds
     2*h_post_n * diff and copies PSUM->SBUF; DMA out.

Self-contained: hardcodes shapes; builds the Bass program once and caches it.
"""

import sys

sys.path.insert(0, "/opt/trn_rl_repo")

from contextlib import ExitStack

import ml_dtypes
import numpy as np

import concourse.bass as bass
import concourse.mybir as mybir
import concourse.tile as tile
from concourse import bacc, bass_utils

F32 = mybir.dt.float32
BF16 = mybir.dt.bfloat16
AF = mybir.ActivationFunctionType
ALU = mybir.AluOpType

T, N, C = 8192, 4, 2048
NCF = N * C  # 8192 flattened features
NFACT = 24
NCORES = 8
P = 128  # partitions / tokens per tile
EPS = float(np.finfo(np.float32).eps)


class _OneActSetBacc(bacc.Bacc):
    """Pin every activation to one table set so the per-tile Ln/Exp mix
    doesn't ping-pong ACT_TABLE_LOADs between sets.

    The (name, funcs) list passed to the insertion pass is positional —
    index == act_func_set_id — so entries other than the chosen set are
    emptied (never chosen) while keeping ids intact. All ACT funcs this
    kernel uses (Square, Ln, Exp, Copy) live in natural_log_exp_and_others.
    """

    _ACT_SET = "natural_log_exp_and_others"

    def insert_act_table_loads(self):
        import concourse.mybir as _mb
        from concourse.hw_specs import get_activation_tables
        import bass_rust as _br

        has_activation = any(
            isinstance(i, _mb.InstActivation)
            for b in self.main_func.blocks
            for i in b.instructions
        )
        if not has_activation:
            return
        tables = []
        for name, funcs in get_activation_tables(self.m.arch).items():
            tables.append((name, funcs if name == self._ACT_SET else set()))
        _br.insert_act_table_loads(self, tables)


def build_program(t_core: int, reps: int = 1, num_devices: int = NCORES):
    nt = t_core // P
    nc = _OneActSetBacc(
        "TRN2", target_bir_lowering=False, debug=False, num_devices=num_devices
    )

    x_d = nc.dram_tensor("x", [t_core, NCF], F32, kind="ExternalInput").ap()
    wallt_d = nc.dram_tensor("wallt", [P, 64, 32], BF16, kind="ExternalInput").ap()
    wp_d = nc.dram_tensor("wp", [P, 16, C], BF16, kind="ExternalInput").ap()
    blayer_d = nc.dram_tensor("blayer", [1, C], BF16, kind="ExternalInput").ap()
    perm_d = nc.dram_tensor("permaug", [NFACT, 17], F32, kind="ExternalInput").ap()
    ab_d = nc.dram_tensor("alphab", [2, 32], F32, kind="ExternalInput").ap()
    idbf_d = nc.dram_tensor("idbf", [P, P], BF16, kind="ExternalInput").ap()
    idf32_d = nc.dram_tensor("idf32", [P, P], F32, kind="ExternalInput").ap()
    out_d = nc.dram_tensor("out", [t_core, NCF], F32, kind="ExternalOutput").ap()

    with tile.TileContext(nc) as tc:
        _build_body(
            tc, nt, reps, x_d, wallt_d, wp_d, blayer_d, perm_d, ab_d,
            idbf_d, idf32_d, out_d,
        )
    nc.compile()
    return nc


def _build_body(
    tc, nt, reps, x_d, wallt_d, wp_d, blayer_d, perm_d, ab_d, idbf_d,
    idf32_d, out_d,
):
    nc = tc.nc
    with ExitStack() as ctx:
        singles = ctx.enter_context(tc.tile_pool(name="singles", bufs=1))
        xnp = ctx.enter_context(tc.tile_pool(name="xnp", bufs=3))
        xtp = ctx.enter_context(tc.tile_pool(name="xtp", bufs=6))
        smalls = ctx.enter_context(tc.tile_pool(name="smalls", bufs=3))
        sqp = ctx.enter_context(tc.tile_pool(name="sqp", bufs=1))
        diagp = ctx.enter_context(tc.tile_pool(name="diagp", bufs=2))
        xgp = ctx.enter_context(tc.tile_pool(name="xgp", bufs=2))
        lip = ctx.enter_context(tc.tile_pool(name="lip", bufs=2))
        ps_small = ctx.enter_context(
            tc.tile_pool(name="ps_small", bufs=2, space="PSUM")
        )
        ps_diff = ctx.enter_context(
            tc.tile_pool(name="ps_diff", bufs=2, space="PSUM")
        )
        ps_mix = ctx.enter_context(tc.tile_pool(name="ps_mix", bufs=4, space="PSUM"))

        # ---- small one-time parameter loads ----
        wp_s = singles.tile([P, 16, C], BF16)  # loaded after tile 0's x below
        walls = singles.tile([P, 64, 32], BF16)
        nc.sync.dma_start(out=walls[:], in_=wallt_d[:])
        perm_s = singles.tile([NFACT, 17], F32)
        nc.sync.dma_start(out=perm_s[:], in_=perm_d[:])
        idbf_s = singles.tile([P, P], BF16)
        nc.sync.dma_start(out=idbf_s[:], in_=idbf_d[:])
        idf32_s = singles.tile([P, P], F32)
        nc.sync.dma_start(out=idf32_s[:], in_=idf32_d[:])
        blb = singles.tile([P, C], BF16)
        nc.gpsimd.dma_start(
            out=blb[:],
            in_=bass.AP(tensor=blayer_d.tensor, offset=blayer_d.offset,
                        ap=[[0, P], [1, C]]),
        )
        alpha_b = singles.tile([P, 32], F32)
        nc.gpsimd.dma_start(
            out=alpha_b[:],
            in_=bass.AP(tensor=ab_d.tensor, offset=ab_d.offset,
                        ap=[[0, P], [1, 32]]),
        )
        bias_b = singles.tile([P, 32], F32)
        nc.gpsimd.dma_start(
            out=bias_b[:],
            in_=bass.AP(tensor=ab_d.tensor, offset=ab_d.offset + 32,
                        ap=[[0, P], [1, 32]]),
        )
        eps_t = singles.tile([P, 1], F32)
        nc.vector.memset(eps_t[:], EPS)

        def stage_load(t):
            """x cast-load, sum-of-squares, inv-rms, xbar transposes."""
            rows = slice(t * P, (t + 1) * P)
            st = {}

            ssqp = smalls.tile([P, N], F32, tag="ssqp", name=f"ssqp{t}")
            xn = xnp.tile([P, NCF], BF16, tag="xn", name=f"xn{t}")
            sqt = sqp.tile([P, C], BF16, tag="sqt", name=f"sqt{t}")
            xTs = []
            for m in range(N):
                # fp32 HBM -> bf16 SBUF cast during the DMA (SWDGE)
                nc.gpsimd.dma_start(
                    out=xn[:, m * C:(m + 1) * C],
                    in_=x_d[rows, m * C:(m + 1) * C],
                )
                # only the free-dim accumulator is consumed; sqt is scratch
                nc.scalar.activation(
                    out=sqt[:], in_=xn[:, m * C:(m + 1) * C],
                    func=AF.Square, accum_out=ssqp[:, m:m + 1],
                )
                xT = xtp.tile([P, 16, P], BF16, tag="xT", name=f"xT{t}_{m}")
                nc.sync.dma_start_transpose(
                    out=xT[:], in_=xn[:, m * C:(m + 1) * C]
                )
                xTs.append(xT)

            ssq = smalls.tile([P, 1], F32, tag="ssq", name=f"ssq{t}")
            nc.vector.tensor_reduce(
                out=ssq[:], in_=ssqp[:], axis=mybir.AxisListType.X, op=ALU.add
            )
            # irms = (mean(x^2) + eps)^-0.5 = exp(-0.5 * ln(ssq/NC + eps))
            lssq = smalls.tile([P, 1], F32, tag="lssq", name=f"lssq{t}")
            nc.scalar.activation(
                out=lssq[:], in_=ssq[:], func=AF.Ln, bias=eps_t[:],
                scale=1.0 / NCF,
            )
            irms = smalls.tile([P, 1], F32, tag="irms", name=f"irms{t}")
            nc.scalar.activation(out=irms[:], in_=lssq[:], func=AF.Exp, scale=-0.5)

            st["xn"] = xn
            st["xTs"] = xTs
            st["irms"] = irms
            st["rows"] = rows
            st["t"] = t
            return st

        def stage_coeff(st):
            """Projection, gate coefficients, li + its transpose, diags."""
            xn = st["xn"]
            xTs = st["xTs"]
            irms = st["irms"]
            t = st["t"]

            proj_p = ps_small.tile([P, 32], F32, tag="pssmall", name=f"prp{t}")
            for m in range(N):
                for kk in range(16):
                    k = m * 16 + kk
                    nc.tensor.matmul(
                        proj_p[:], xTs[m][:, kk, :], walls[:, k, :],
                        start=(k == 0), stop=(k == 63),
                    )

            # scaled = (proj * irms) * alpha + bias; alpha/bias negated on 0:8
            scaled = smalls.tile([P, 32], F32, tag="scaled", name=f"scl{t}")
            nc.vector.scalar_tensor_tensor(
                out=scaled[:], in0=proj_p[:], scalar=irms[:], in1=alpha_b[:],
                op0=ALU.mult, op1=ALU.mult,
            )
            nc.vector.tensor_add(scaled[:], scaled[:], bias_b[:])

            # eall: cols 0:8 = exp(-z) (sigmoid input), cols 8:32 = softmax exps
            eall = smalls.tile([P, 32], F32, tag="eall", name=f"eall{t}")
            nc.scalar.activation(out=eall[:], in_=scaled[:], func=AF.Exp)

            # h = 1 / (1 + exp(-z)) for the 8 sigmoid outputs
            hden = smalls.tile([P, 8], F32, tag="hden", name=f"hden{t}")
            nc.vector.tensor_scalar_add(hden[:], eall[:, 0:8], 1.0)
            hps = smalls.tile([P, 8], F32, tag="hps", name=f"hps{t}")
            nc.vector.reciprocal(out=hps[:], in_=hden[:])

            # li early: libf = sum_m h_pre_m * x_m (DVE, bf16), then its
            # xbar transpose fires while the rest of the chain runs.
            libf = lip.tile([P, C], BF16, tag="libf", name=f"libf{t}")
            nc.vector.tensor_scalar_mul(libf[:], xn[:, 0:C], hps[:, 0:1])
            for m in range(1, N):
                nc.vector.scalar_tensor_tensor(
                    out=libf[:], in0=xn[:, m * C:(m + 1) * C],
                    scalar=hps[:, m:m + 1], in1=libf[:],
                    op0=ALU.mult, op1=ALU.add,
                )
            liT = lip.tile([P, 16, P], BF16, tag="liT", name=f"liT{t}")
            nc.sync.dma_start_transpose(out=liT[:], in_=libf[:])

            expsT_p = ps_small.tile([NFACT, P], F32, tag="pssmall", name=f"exT{t}")
            nc.tensor.transpose(expsT_p[:], eall[:, 8:32], idf32_s[:])
            expsT_s = smalls.tile([NFACT, P], F32, tag="expsT_s", name=f"exs{t}")
            nc.scalar.activation(out=expsT_s[:], in_=expsT_p[:], func=AF.Copy)

            haug_p = ps_small.tile([P, 17], F32, tag="pssmall", name=f"hgp{t}")
            nc.tensor.matmul(
                haug_p[:], expsT_s[:], perm_s[:], start=True, stop=True
            )
            hd = smalls.tile([P, 17], F32, tag="hd", name=f"hd{t}")
            nc.scalar.activation(out=hd[:], in_=haug_p[:], func=AF.Copy)

            dinv = smalls.tile([P, 1], F32, tag="dinv", name=f"dinv{t}")
            nc.vector.reciprocal(out=dinv[:], in_=hd[:, 16:17])

            # coeffs cols 0:16 = normalized H (col 4m+n = H[n,m]);
            # 16:20 = 2*h_post
            coeffs = smalls.tile([P, 20], F32, tag="coeffs", name=f"co{t}")
            nc.vector.tensor_scalar_mul(coeffs[:, 0:16], hd[:, 0:16], dinv[:])
            nc.vector.tensor_scalar_mul(coeffs[:, 16:20], hps[:, 4:8], 2.0)

            # diags: j=4m+n -> H[n,m] for the mixing matmuls
            diags = diagp.tile([P, 16, P], BF16, tag="diags", name=f"dg{t}")
            for j in range(16):
                nc.vector.tensor_scalar_mul(
                    diags[:, j, :], idbf_s[:], coeffs[:, j:j + 1]
                )

            st["diags"] = diags
            st["coeffs"] = coeffs
            st["liT"] = liT
            return st

        def stage_b(st):
            """diff = liT.T @ (W.T - I) + b, mixing, store."""
            xn = st["xn"]
            diags = st["diags"]
            rows = st["rows"]
            coeffs = st["coeffs"]
            liT = st["liT"]
            t = st["t"]

            diffbf = lip.tile([P, C], BF16, tag="diffbf", name=f"diffbf{t}")
            for q in range(4):
                cs = slice(q * 512, (q + 1) * 512)
                diff_p = ps_diff.tile([P, 512], F32, tag="diff")
                for k in range(16):
                    nc.tensor.matmul(
                        diff_p[:], liT[:, k, :], wp_s[:, k, cs],
                        start=(k == 0), stop=(k == 15),
                    )
                # diffbf = diff + b_layer (broadcast), cast to bf16
                nc.vector.scalar_tensor_tensor(
                    out=diffbf[:, cs], in0=diff_p[:], scalar=1.0,
                    in1=blb[:, cs], op0=ALU.bypass, op1=ALU.add,
                )

            # ---- mixing: out_n = sum_m diag(H[n,m]) @ x_m + h_post2_n*diff
            for n in range(N):
                outsb = xgp.tile([P, C], F32, tag="outsb", name=f"ou{t}_{n}")
                for cc in range(4):
                    cs = slice(cc * 512, (cc + 1) * 512)
                    mix_p = ps_mix.tile([P, 512], F32, tag="mix",
                                        name=f"mx{t}_{n}_{cc}")
                    for src_ in range(N):
                        nc.tensor.matmul(
                            mix_p[:], diags[:, 4 * src_ + n, :],
                            xn[:, src_ * C + cc * 512: src_ * C + (cc + 1) * 512],
                            start=(src_ == 0), stop=(src_ == 3),
                        )
                    nc.vector.scalar_tensor_tensor(
                        out=outsb[:, cs], in0=diffbf[:, cs],
                        scalar=coeffs[:, 16 + n:17 + n], in1=mix_p[:],
                        op0=ALU.mult, op1=ALU.add,
                    )
                nc.sync.dma_start(
                    out=out_d[rows, n * C:(n + 1) * C], in_=outsb[:]
                )

        # ---- software-pipelined emission ----
        # Per iteration: loads(t+1) first (DMA queues fill early), then the
        # heavy PE work of tile t (diff+mix), then tile t+1's coefficient
        # chain. Keeps ready work at each engine FIFO's head: tile t+1's
        # proj/diags (gated on DMA) never sit ahead of tile t's diff/mix.
        first = True
        pending = None  # tile with coeffs done, stage_b outstanding
        for rep in range(reps):
            for t in range(nt):
                ld = stage_load(t)
                if first:
                    # defer the big weight load until after tile 0's x DMAs
                    nc.sync.dma_start(out=wp_s[:], in_=wp_d[:])
                    first = False
                if pending is not None:
                    stage_b(pending)
                pending = stage_coeff(ld)
        stage_b(pending)


def prep_params(inputs):
    """Host-side parameter preprocessing shared by all cores."""
    bf = ml_dtypes.bfloat16
    W_all = np.asarray(inputs["W_all"], np.float32)
    W_layer = np.asarray(inputs["W_layer"], np.float32)
    b_all = np.asarray(inputs["b_all"], np.float32)
    b_layer = np.asarray(inputs["b_layer"], np.float32)
    perm_mat = np.asarray(inputs["perm_mat"], np.float32)
    a_pre = float(np.asarray(inputs["alpha_pre"]).reshape(-1)[0])
    a_post = float(np.asarray(inputs["alpha_post"]).reshape(-1)[0])
    a_res = float(np.asarray(inputs["alpha_res"]).reshape(-1)[0])

    wallt = np.ascontiguousarray(
        W_all.T.astype(bf).reshape(64, P, 32).transpose(1, 0, 2)
    )
    wp = (np.ascontiguousarray(W_layer.T) - np.eye(C, dtype=np.float32))
    wp = np.ascontiguousarray(wp.astype(bf).reshape(16, P, C).transpose(1, 0, 2))
    blayer = b_layer.astype(bf).reshape(1, C)
    # perm_aug columns in m-major order: col 4m+n = perm_mat[:, n*4+m]; col 16 = 1
    perm_aug = np.zeros((NFACT, 17), np.float32)
    perm_aug[:, :16] = perm_mat.reshape(NFACT, N, N).transpose(0, 2, 1).reshape(
        NFACT, 16
    )
    perm_aug[:, 16] = 1.0
    # cols 0:8 negated: eall = exp(-(alpha*p + b)) there, for sigmoid via 1/(1+u)
    alphab = np.zeros((2, 32), np.float32)
    alphab[0, 0:4] = -a_pre
    alphab[0, 4:8] = -a_post
    alphab[0, 8:32] = a_res
    alphab[1, 0:4] = -b_all[0:4]
    alphab[1, 4:8] = -b_all[4:8]
    alphab[1, 8:32] = b_all[8:32]
    idbf = np.eye(P, dtype=np.float32).astype(bf)
    idf32 = np.eye(P, dtype=np.float32)
    return {
        "wallt": wallt, "wp": wp, "blayer": blayer,
        "permaug": perm_aug, "alphab": alphab, "idbf": idbf, "idf32": idf32,
    }


_PROGRAM_CACHE = {}


def get_program(t_core):
    if t_core not in _PROGRAM_CACHE:
        _PROGRAM_CACHE[t_core] = build_program(t_core)
    return _PROGRAM_CACHE[t_core]


def run(inputs, trace=False):
    x = np.asarray(inputs["x_streams"], np.float32).reshape(T, NCF)
    params = prep_params(inputs)
    t_core = T // NCORES
    nc = get_program(t_core)
    in_maps = []
    for c in range(NCORES):
        m = dict(params)
        m["x"] = np.ascontiguousarray(x[c * t_core:(c + 1) * t_core])
        in_maps.append(m)
    res = bass_utils.run_bass_kernel_spmd(
        nc, in_maps, core_ids=list(range(NCORES)), trace=trace
    )
    out = np.concatenate([r["out"] for r in res.results], axis=0)
    return out.reshape(T, N, C).astype(np.float32), res


def kernel(**inputs) -> np.ndarray:
    out, _ = run(inputs)
    return out


def bench_reps(inputs, reps=5, calls=7):
    """Single-core timing: diff a reps-unrolled program against reps=1.

    Inputs are device-resident; each call is one NEFF execution, so the
    difference isolates (reps-1) kernel-body repetitions.
    """
    import time as _time

    import jax

    from concourse import bass2jax
    from concourse import mybir as _mb

    x = np.asarray(inputs["x_streams"], np.float32).reshape(T, NCF)
    params = prep_params(inputs)
    t_core = T // NCORES
    bass2jax.install_neuronx_cc_hook()

    results = {}
    for r in (1, reps):
        nc = build_program(t_core, reps=r, num_devices=1)
        partition_name = (
            nc.partition_id_tensor.name if nc.partition_id_tensor else None
        )
        in_names, out_names, out_avals, zero_outs = [], [], [], []
        for alloc in nc.m.functions[0].allocations:
            if not isinstance(alloc, _mb.MemoryLocationSet):
                continue
            name = alloc.memorylocations[0].name
            if alloc.kind == "ExternalInput":
                if name != partition_name:
                    in_names.append(name)
            elif alloc.kind == "ExternalOutput":
                out_names.append(name)
                shape = tuple(alloc.tensor_shape)
                dtype = _mb.dt.np(alloc.dtype)
                out_avals.append(jax.core.ShapedArray(shape, dtype))
                zero_outs.append(np.zeros(shape, dtype))
        bind_names = list(in_names) + list(out_names)
        if partition_name is not None:
            bind_names.append(partition_name)

        def _body(*flat, _nc=nc, _bind=tuple(bind_names),
                  _outn=tuple(out_names), _avals=tuple(out_avals),
                  _pn=partition_name):
            operands = list(flat)
            if _pn is not None:
                operands.append(bass2jax.partition_id_tensor())
            return tuple(bass2jax._bass_exec_p.bind(
                *operands, out_avals=_avals, in_names=_bind, out_names=_outn,
                lowering_input_output_aliases=(),
                sim_require_finite=True, sim_require_nnan=True, nc=_nc,
            ))

        m = dict(params)
        m["x"] = np.ascontiguousarray(x[:t_core])
        dev = jax.devices()[0]
        args = [jax.device_put(np.asarray(m[n]), dev) for n in in_names]
        args += [jax.device_put(z, dev) for z in zero_outs]
        fn = jax.jit(_body)
        outs = fn(*args)
        jax.block_until_ready(outs)
        best = None
        for _ in range(calls):
            t0 = _time.perf_counter()
            outs = fn(*args)
            jax.block_until_ready(outs)
            dt = _time.perf_counter() - t0
            best = dt if best is None else min(best, dt)
        results[r] = best
        print(f"  reps={r}: best call {best*1e3:.3f} ms")
    ns = (results[reps] - results[1]) / (reps - 1) * 1e9
    return ns


def bench(inputs, iters=8):
    """Time `iters` chained kernel executions on HW inside one jit.

    Outputs are fed back as the (normally zero-initialized) output buffers of
    the next iteration; the kernel overwrites every output element, so values
    stay correct and the data dependency serializes executions on-device.
    Returns (ns_per_iter, out_of_last_iter).
    """
    import jax
    from jax.sharding import Mesh, PartitionSpec
    from jax.experimental.shard_map import shard_map
    import time as _time
    from concourse import bass2jax, mybir as _mb

    x = np.asarray(inputs["x_streams"], np.float32).reshape(T, NCF)
    params = prep_params(inputs)
    t_core = T // NCORES
    nc = get_program(t_core)
    bass2jax.install_neuronx_cc_hook()

    partition_name = (
        nc.partition_id_tensor.name if nc.partition_id_tensor else None
    )
    in_names, out_names, out_avals, zero_outs = [], [], [], []
    for alloc in nc.m.functions[0].allocations:
        if not isinstance(alloc, _mb.MemoryLocationSet):
            continue
        name = alloc.memorylocations[0].name
        if alloc.kind == "ExternalInput":
            if name != partition_name:
                in_names.append(name)
        elif alloc.kind == "ExternalOutput":
            out_names.append(name)
            shape = tuple(alloc.tensor_shape)
            dtype = _mb.dt.np(alloc.dtype)
            out_avals.append(jax.core.ShapedArray(shape, dtype))
            zero_outs.append(np.zeros(shape, dtype))
    n_params = len(in_names)

    bind_names = list(in_names) + list(out_names)
    if partition_name is not None:
        bind_names.append(partition_name)

    def body_once(args, outs):
        operands = list(args) + list(outs)
        if partition_name is not None:
            operands.append(bass2jax.partition_id_tensor())
        res = bass2jax._bass_exec_p.bind(
            *operands,
            out_avals=tuple(out_avals),
            in_names=tuple(bind_names),
            out_names=tuple(out_names),
            lowering_input_output_aliases=(),
            sim_require_finite=True,
            sim_require_nnan=True,
            nc=nc,
        )
        return tuple(res)

    def chain(k):
        def _body(*flat):
            args = flat[:n_params]
            outs = flat[n_params:]
            for _ in range(k):
                outs = body_once(args, outs)
            return outs
        return _body

    devices = jax.devices()[:NCORES]
    mesh = Mesh(np.asarray(devices), ("core",))
    in_specs = (PartitionSpec("core"),) * (n_params + len(out_names))
    out_specs = (PartitionSpec("core"),) * len(out_names)

    per_core = []
    for c in range(NCORES):
        m = dict(params)
        m["x"] = np.ascontiguousarray(x[c * t_core:(c + 1) * t_core])
        per_core.append([np.asarray(m[n]) for n in in_names])
    concat_in = [
        np.concatenate([per_core[c][i] for c in range(NCORES)], axis=0)
        for i in range(n_params)
    ]
    concat_zeros = [
        np.zeros((NCORES * z.shape[0], *z.shape[1:]), z.dtype) for z in zero_outs
    ]

    times = {}
    for k in (1, 1 + iters):
        fn = jax.jit(
            shard_map(chain(k), mesh=mesh, in_specs=in_specs,
                      out_specs=out_specs, check_rep=False)
        )
        out_arrs = fn(*concat_in, *concat_zeros)  # compile+warm
        jax.block_until_ready(out_arrs)
        reps = 3
        best = None
        for _ in range(reps):
            t0 = _time.perf_counter()
            out_arrs = fn(*concat_in, *concat_zeros)
            jax.block_until_ready(out_arrs)
            dt = _time.perf_counter() - t0
            best = dt if best is None else min(best, dt)
        times[k] = best
    ns = (times[1 + iters] - times[1]) / iters * 1e9
    out = np.asarray(out_arrs[0]).reshape(NCORES, t_core, NCF).reshape(T, N, C)
    return ns, out



# revision 21
# speedup vs baseline: 1.0148x; 1.0148x over previous
"""MHCLiteBlock Trainium2 kernel.

Data-parallel over T across 8 NeuronCores (1024 tokens/core); all params
replicated. Per core, per 128-token tile:

  1. SWDGE cast-DMA: x fp32 HBM -> xn bf16 SBUF (4 chunks).
  2. ACT Square+accum on xn -> ssq; irms = exp(-0.5*ln(ssq/NC + eps))
     (ln/exp keep every ACT transcendental in ONE table set).
  3. DMA-xbar-transpose xn -> xT chunks [128c, 128t].
  4. proj (PE): proj[t, 32] = sum_k xT_k.T @ wallT_k directly in [t, .] layout.
  5. scaled = (proj * irms) * alpha + bias with alpha/bias negated on cols 0:8;
     eall = Exp(scaled): cols 0:8 = exp(-z) -> sigmoid via DVE 1/(1+u);
     cols 8:32 = softmax numerators. Soft permutation H via perm_aug matmul.
  6. li (DVE): libf = sum_m h_pre_m * x_m, bf16.
  7. M2 (PE): diff = liT.T @ (W_layer.T - I) + b_layer.
  8. Mixing (PE): out_n = sum_m diag(H[n,m]) @ x_m; DVE combine adds
     2*h_post_n * diff and copies PSUM->SBUF; DMA out.

Self-contained: hardcodes shapes; builds the Bass program once and caches it.
"""

import sys

sys.path.insert(0, "/opt/trn_rl_repo")

from contextlib import ExitStack

import ml_dtypes
import numpy as np

import concourse.bass as bass
import concourse.mybir as mybir
import concourse.tile as tile
from concourse import bacc, bass_utils

F32 = mybir.dt.float32
BF16 = mybir.dt.bfloat16
AF = mybir.ActivationFunctionType
ALU = mybir.AluOpType

T, N, C = 8192, 4, 2048
NCF = N * C  # 8192 flattened features
NFACT = 24
NCORES = 8
P = 128  # partitions / tokens per tile
EPS = float(np.finfo(np.float32).eps)


class _OneActSetBacc(bacc.Bacc):
    """Pin every activation to one table set so the per-tile Ln/Exp mix
    doesn't ping-pong ACT_TABLE_LOADs between sets.

    The (name, funcs) list passed to the insertion pass is positional —
    index == act_func_set_id — so entries other than the chosen set are
    emptied (never chosen) while keeping ids intact. All ACT funcs this
    kernel uses (Square, Ln, Exp, Copy) live in natural_log_exp_and_others.
    """

    _ACT_SET = "natural_log_exp_and_others"

    def insert_act_table_loads(self):
        import concourse.mybir as _mb
        from concourse.hw_specs import get_activation_tables
        import bass_rust as _br

        has_activation = any(
            isinstance(i, _mb.InstActivation)
            for b in self.main_func.blocks
            for i in b.instructions
        )
        if not has_activation:
            return
        tables = []
        for name, funcs in get_activation_tables(self.m.arch).items():
            tables.append((name, funcs if name == self._ACT_SET else set()))
        _br.insert_act_table_loads(self, tables)


def build_program(t_core: int, reps: int = 1, num_devices: int = NCORES):
    nt = t_core // P
    nc = _OneActSetBacc(
        "TRN2", target_bir_lowering=False, debug=False, num_devices=num_devices
    )

    x_d = nc.dram_tensor("x", [t_core, NCF], F32, kind="ExternalInput").ap()
    wallt_d = nc.dram_tensor("wallt", [P, 64, 32], BF16, kind="ExternalInput").ap()
    wp_d = nc.dram_tensor("wp", [P, 16, C], BF16, kind="ExternalInput").ap()
    blayer_d = nc.dram_tensor("blayer", [1, C], BF16, kind="ExternalInput").ap()
    perm_d = nc.dram_tensor("permaug", [NFACT, 17], F32, kind="ExternalInput").ap()
    ab_d = nc.dram_tensor("alphab", [2, 32], F32, kind="ExternalInput").ap()
    idbf_d = nc.dram_tensor("idbf", [P, P], BF16, kind="ExternalInput").ap()
    idf32_d = nc.dram_tensor("idf32", [P, P], F32, kind="ExternalInput").ap()
    out_d = nc.dram_tensor("out", [t_core, NCF], F32, kind="ExternalOutput").ap()

    with tile.TileContext(nc) as tc:
        _build_body(
            tc, nt, reps, x_d, wallt_d, wp_d, blayer_d, perm_d, ab_d,
            idbf_d, idf32_d, out_d,
        )
    nc.compile()
    return nc


def _build_body(
    tc, nt, reps, x_d, wallt_d, wp_d, blayer_d, perm_d, ab_d, idbf_d,
    idf32_d, out_d,
):
    nc = tc.nc
    with ExitStack() as ctx:
        singles = ctx.enter_context(tc.tile_pool(name="singles", bufs=1))
        xnp = ctx.enter_context(tc.tile_pool(name="xnp", bufs=3))
        xtp = ctx.enter_context(tc.tile_pool(name="xtp", bufs=6))
        smalls = ctx.enter_context(tc.tile_pool(name="smalls", bufs=3))
        sqp = ctx.enter_context(tc.tile_pool(name="sqp", bufs=1))
        diagp = ctx.enter_context(tc.tile_pool(name="diagp", bufs=2))
        xgp = ctx.enter_context(tc.tile_pool(name="xgp", bufs=2))
        lip = ctx.enter_context(tc.tile_pool(name="lip", bufs=2))
        ps_small = ctx.enter_context(
            tc.tile_pool(name="ps_small", bufs=1, space="PSUM")
        )
        ps_diff = ctx.enter_context(
            tc.tile_pool(name="ps_diff", bufs=2, space="PSUM")
        )
        ps_mix = ctx.enter_context(tc.tile_pool(name="ps_mix", bufs=5, space="PSUM"))

        # ---- small one-time parameter loads ----
        wp_s = singles.tile([P, 16, C], BF16)  # loaded after tile 0's x below
        walls = singles.tile([P, 64, 32], BF16)
        nc.sync.dma_start(out=walls[:], in_=wallt_d[:])
        perm_s = singles.tile([NFACT, 17], F32)
        nc.sync.dma_start(out=perm_s[:], in_=perm_d[:])
        idbf_s = singles.tile([P, P], BF16)
        nc.sync.dma_start(out=idbf_s[:], in_=idbf_d[:])
        idf32_s = singles.tile([P, P], F32)
        nc.sync.dma_start(out=idf32_s[:], in_=idf32_d[:])
        blb = singles.tile([P, C], BF16)
        nc.gpsimd.dma_start(
            out=blb[:],
            in_=bass.AP(tensor=blayer_d.tensor, offset=blayer_d.offset,
                        ap=[[0, P], [1, C]]),
        )
        alpha_b = singles.tile([P, 32], F32)
        nc.gpsimd.dma_start(
            out=alpha_b[:],
            in_=bass.AP(tensor=ab_d.tensor, offset=ab_d.offset,
                        ap=[[0, P], [1, 32]]),
        )
        bias_b = singles.tile([P, 32], F32)
        nc.gpsimd.dma_start(
            out=bias_b[:],
            in_=bass.AP(tensor=ab_d.tensor, offset=ab_d.offset + 32,
                        ap=[[0, P], [1, 32]]),
        )
        eps_t = singles.tile([P, 1], F32)
        nc.vector.memset(eps_t[:], EPS)

        def stage_load(t):
            """x cast-load, sum-of-squares, inv-rms, xbar transposes."""
            rows = slice(t * P, (t + 1) * P)
            st = {}

            ssqp = smalls.tile([P, N], F32, tag="ssqp", name=f"ssqp{t}")
            xn = xnp.tile([P, NCF], BF16, tag="xn", name=f"xn{t}")
            sqt = sqp.tile([P, C], BF16, tag="sqt", name=f"sqt{t}")
            xTs = []
            for m in range(N):
                # fp32 HBM -> bf16 SBUF cast during the DMA (SWDGE)
                nc.gpsimd.dma_start(
                    out=xn[:, m * C:(m + 1) * C],
                    in_=x_d[rows, m * C:(m + 1) * C],
                )
                # only the free-dim accumulator is consumed; sqt is scratch
                nc.scalar.activation(
                    out=sqt[:], in_=xn[:, m * C:(m + 1) * C],
                    func=AF.Square, accum_out=ssqp[:, m:m + 1],
                )
                xT = xtp.tile([P, 16, P], BF16, tag="xT", name=f"xT{t}_{m}")
                nc.sync.dma_start_transpose(
                    out=xT[:], in_=xn[:, m * C:(m + 1) * C]
                )
                xTs.append(xT)

            ssq = smalls.tile([P, 1], F32, tag="ssq", name=f"ssq{t}")
            nc.vector.tensor_reduce(
                out=ssq[:], in_=ssqp[:], axis=mybir.AxisListType.X, op=ALU.add
            )
            # irms = (mean(x^2) + eps)^-0.5 = exp(-0.5 * ln(ssq/NC + eps))
            lssq = smalls.tile([P, 1], F32, tag="lssq", name=f"lssq{t}")
            nc.scalar.activation(
                out=lssq[:], in_=ssq[:], func=AF.Ln, bias=eps_t[:],
                scale=1.0 / NCF,
            )
            irms = smalls.tile([P, 1], F32, tag="irms", name=f"irms{t}")
            nc.scalar.activation(out=irms[:], in_=lssq[:], func=AF.Exp, scale=-0.5)

            st["xn"] = xn
            st["xTs"] = xTs
            st["irms"] = irms
            st["rows"] = rows
            st["t"] = t
            return st

        def stage_coeff(st):
            """Projection, gate coefficients, li + its transpose, diags."""
            xn = st["xn"]
            xTs = st["xTs"]
            irms = st["irms"]
            t = st["t"]

            proj_p = ps_small.tile([P, 32], F32, tag="pssmall", name=f"prp{t}")
            for m in range(N):
                for kk in range(16):
                    k = m * 16 + kk
                    nc.tensor.matmul(
                        proj_p[:], xTs[m][:, kk, :], walls[:, k, :],
                        start=(k == 0), stop=(k == 63),
                    )

            # scaled = (proj * irms) * alpha + bias; alpha/bias negated on 0:8
            scaled = smalls.tile([P, 32], F32, tag="scaled", name=f"scl{t}")
            nc.vector.scalar_tensor_tensor(
                out=scaled[:], in0=proj_p[:], scalar=irms[:], in1=alpha_b[:],
                op0=ALU.mult, op1=ALU.mult,
            )
            nc.vector.tensor_add(scaled[:], scaled[:], bias_b[:])

            # eall: cols 0:8 = exp(-z) (sigmoid input), cols 8:32 = softmax exps
            eall = smalls.tile([P, 32], F32, tag="eall", name=f"eall{t}")
            nc.scalar.activation(out=eall[:], in_=scaled[:], func=AF.Exp)

            # h = 1 / (1 + exp(-z)) for the 8 sigmoid outputs
            hden = smalls.tile([P, 8], F32, tag="hden", name=f"hden{t}")
            nc.vector.tensor_scalar_add(hden[:], eall[:, 0:8], 1.0)
            hps = smalls.tile([P, 8], F32, tag="hps", name=f"hps{t}")
            nc.vector.reciprocal(out=hps[:], in_=hden[:])

            # li early: libf = sum_m h_pre_m * x_m (DVE, bf16), then its
            # xbar transpose fires while the rest of the chain runs.
            libf = lip.tile([P, C], BF16, tag="libf", name=f"libf{t}")
            nc.vector.tensor_scalar_mul(libf[:], xn[:, 0:C], hps[:, 0:1])
            for m in range(1, N):
                nc.vector.scalar_tensor_tensor(
                    out=libf[:], in0=xn[:, m * C:(m + 1) * C],
                    scalar=hps[:, m:m + 1], in1=libf[:],
                    op0=ALU.mult, op1=ALU.add,
                )
            liT = lip.tile([P, 16, P], BF16, tag="liT", name=f"liT{t}")
            nc.sync.dma_start_transpose(out=liT[:], in_=libf[:])

            expsT_p = ps_small.tile([NFACT, P], F32, tag="pssmall", name=f"exT{t}")
            nc.tensor.transpose(expsT_p[:], eall[:, 8:32], idf32_s[:])
            expsT_s = smalls.tile([NFACT, P], F32, tag="expsT_s", name=f"exs{t}")
            nc.scalar.activation(out=expsT_s[:], in_=expsT_p[:], func=AF.Copy)

            haug_p = ps_small.tile([P, 17], F32, tag="pssmall", name=f"hgp{t}")
            nc.tensor.matmul(
                haug_p[:], expsT_s[:], perm_s[:], start=True, stop=True
            )
            hd = smalls.tile([P, 17], F32, tag="hd", name=f"hd{t}")
            nc.scalar.activation(out=hd[:], in_=haug_p[:], func=AF.Copy)

            dinv = smalls.tile([P, 1], F32, tag="dinv", name=f"dinv{t}")
            nc.vector.reciprocal(out=dinv[:], in_=hd[:, 16:17])

            # coeffs cols 0:16 = normalized H (col 4m+n = H[n,m]);
            # 16:20 = 2*h_post
            coeffs = smalls.tile([P, 20], F32, tag="coeffs", name=f"co{t}")
            nc.vector.tensor_scalar_mul(coeffs[:, 0:16], hd[:, 0:16], dinv[:])
            nc.vector.tensor_scalar_mul(coeffs[:, 16:20], hps[:, 4:8], 2.0)

            # diags: j=4m+n -> H[n,m] for the mixing matmuls
            diags = diagp.tile([P, 16, P], BF16, tag="diags", name=f"dg{t}")
            for j in range(16):
                nc.vector.tensor_scalar_mul(
                    diags[:, j, :], idbf_s[:], coeffs[:, j:j + 1]
                )

            st["diags"] = diags
            st["coeffs"] = coeffs
            st["liT"] = liT
            return st

        def stage_b(st):
            """diff = liT.T @ (W.T - I) + b, mixing, store."""
            xn = st["xn"]
            diags = st["diags"]
            rows = st["rows"]
            coeffs = st["coeffs"]
            liT = st["liT"]
            t = st["t"]

            diffbf = lip.tile([P, C], BF16, tag="diffbf", name=f"diffbf{t}")
            for q in range(4):
                cs = slice(q * 512, (q + 1) * 512)
                diff_p = ps_diff.tile([P, 512], F32, tag="diff")
                for k in range(16):
                    nc.tensor.matmul(
                        diff_p[:], liT[:, k, :], wp_s[:, k, cs],
                        start=(k == 0), stop=(k == 15),
                    )
                # diffbf = diff + b_layer (broadcast), cast to bf16
                nc.vector.scalar_tensor_tensor(
                    out=diffbf[:, cs], in0=diff_p[:], scalar=1.0,
                    in1=blb[:, cs], op0=ALU.bypass, op1=ALU.add,
                )

            # ---- mixing: out_n = sum_m diag(H[n,m]) @ x_m + h_post2_n*diff
            for n in range(N):
                outsb = xgp.tile([P, C], F32, tag="outsb", name=f"ou{t}_{n}")
                for cc in range(4):
                    cs = slice(cc * 512, (cc + 1) * 512)
                    mix_p = ps_mix.tile([P, 512], F32, tag="mix",
                                        name=f"mx{t}_{n}_{cc}")
                    for src_ in range(N):
                        nc.tensor.matmul(
                            mix_p[:], diags[:, 4 * src_ + n, :],
                            xn[:, src_ * C + cc * 512: src_ * C + (cc + 1) * 512],
                            start=(src_ == 0), stop=(src_ == 3),
                        )
                    nc.vector.scalar_tensor_tensor(
                        out=outsb[:, cs], in0=diffbf[:, cs],
                        scalar=coeffs[:, 16 + n:17 + n], in1=mix_p[:],
                        op0=ALU.mult, op1=ALU.add,
                    )
                nc.sync.dma_start(
                    out=out_d[rows, n * C:(n + 1) * C], in_=outsb[:]
                )

        # ---- software-pipelined emission ----
        # Per iteration: loads(t+1) first (DMA queues fill early), then the
        # heavy PE work of tile t (diff+mix), then tile t+1's coefficient
        # chain. Keeps ready work at each engine FIFO's head: tile t+1's
        # proj/diags (gated on DMA) never sit ahead of tile t's diff/mix.
        first = True
        pending = None  # tile with coeffs done, stage_b outstanding
        for rep in range(reps):
            for t in range(nt):
                ld = stage_load(t)
                if first:
                    # defer the big weight load until after tile 0's x DMAs
                    nc.sync.dma_start(out=wp_s[:], in_=wp_d[:])
                    first = False
                if pending is not None:
                    stage_b(pending)
                pending = stage_coeff(ld)
        stage_b(pending)


def prep_params(inputs):
    """Host-side parameter preprocessing shared by all cores."""
    bf = ml_dtypes.bfloat16
    W_all = np.asarray(inputs["W_all"], np.float32)
    W_layer = np.asarray(inputs["W_layer"], np.float32)
    b_all = np.asarray(inputs["b_all"], np.float32)
    b_layer = np.asarray(inputs["b_layer"], np.float32)
    perm_mat = np.asarray(inputs["perm_mat"], np.float32)
    a_pre = float(np.asarray(inputs["alpha_pre"]).reshape(-1)[0])
    a_post = float(np.asarray(inputs["alpha_post"]).reshape(-1)[0])
    a_res = float(np.asarray(inputs["alpha_res"]).reshape(-1)[0])

    wallt = np.ascontiguousarray(
        W_all.T.astype(bf).reshape(64, P, 32).transpose(1, 0, 2)
    )
    wp = (np.ascontiguousarray(W_layer.T) - np.eye(C, dtype=np.float32))
    wp = np.ascontiguousarray(wp.astype(bf).reshape(16, P, C).transpose(1, 0, 2))
    blayer = b_layer.astype(bf).reshape(1, C)
    # perm_aug columns in m-major order: col 4m+n = perm_mat[:, n*4+m]; col 16 = 1
    perm_aug = np.zeros((NFACT, 17), np.float32)
    perm_aug[:, :16] = perm_mat.reshape(NFACT, N, N).transpose(0, 2, 1).reshape(
        NFACT, 16
    )
    perm_aug[:, 16] = 1.0
    # cols 0:8 negated: eall = exp(-(alpha*p + b)) there, for sigmoid via 1/(1+u)
    alphab = np.zeros((2, 32), np.float32)
    alphab[0, 0:4] = -a_pre
    alphab[0, 4:8] = -a_post
    alphab[0, 8:32] = a_res
    alphab[1, 0:4] = -b_all[0:4]
    alphab[1, 4:8] = -b_all[4:8]
    alphab[1, 8:32] = b_all[8:32]
    idbf = np.eye(P, dtype=np.float32).astype(bf)
    idf32 = np.eye(P, dtype=np.float32)
    return {
        "wallt": wallt, "wp": wp, "blayer": blayer,
        "permaug": perm_aug, "alphab": alphab, "idbf": idbf, "idf32": idf32,
    }


_PROGRAM_CACHE = {}


def get_program(t_core):
    if t_core not in _PROGRAM_CACHE:
        _PROGRAM_CACHE[t_core] = build_program(t_core)
    return _PROGRAM_CACHE[t_core]


def run(inputs, trace=False):
    x = np.asarray(inputs["x_streams"], np.float32).reshape(T, NCF)
    params = prep_params(inputs)
    t_core = T // NCORES
    nc = get_program(t_core)
    in_maps = []
    for c in range(NCORES):
        m = dict(params)
        m["x"] = np.ascontiguousarray(x[c * t_core:(c + 1) * t_core])
        in_maps.append(m)
    res = bass_utils.run_bass_kernel_spmd(
        nc, in_maps, core_ids=list(range(NCORES)), trace=trace
    )
    out = np.concatenate([r["out"] for r in res.results], axis=0)
    return out.reshape(T, N, C).astype(np.float32), res


def kernel(**inputs) -> np.ndarray:
    out, _ = run(inputs)
    return out


def bench_reps(inputs, reps=5, calls=7):
    """Single-core timing: diff a reps-unrolled program against reps=1.

    Inputs are device-resident; each call is one NEFF execution, so the
    difference isolates (reps-1) kernel-body repetitions.
    """
    import time as _time

    import jax

    from concourse import bass2jax
    from concourse import mybir as _mb

    x = np.asarray(inputs["x_streams"], np.float32).reshape(T, NCF)
    params = prep_params(inputs)
    t_core = T // NCORES
    bass2jax.install_neuronx_cc_hook()

    results = {}
    for r in (1, reps):
        nc = build_program(t_core, reps=r, num_devices=1)
        partition_name = (
            nc.partition_id_tensor.name if nc.partition_id_tensor else None
        )
        in_names, out_names, out_avals, zero_outs = [], [], [], []
        for alloc in nc.m.functions[0].allocations:
            if not isinstance(alloc, _mb.MemoryLocationSet):
                continue
            name = alloc.memorylocations[0].name
            if alloc.kind == "ExternalInput":
                if name != partition_name:
                    in_names.append(name)
            elif alloc.kind == "ExternalOutput":
                out_names.append(name)
                shape = tuple(alloc.tensor_shape)
                dtype = _mb.dt.np(alloc.dtype)
                out_avals.append(jax.core.ShapedArray(shape, dtype))
                zero_outs.append(np.zeros(shape, dtype))
        bind_names = list(in_names) + list(out_names)
        if partition_name is not None:
            bind_names.append(partition_name)

        def _body(*flat, _nc=nc, _bind=tuple(bind_names),
                  _outn=tuple(out_names), _avals=tuple(out_avals),
                  _pn=partition_name):
            operands = list(flat)
            if _pn is not None:
                operands.append(bass2jax.partition_id_tensor())
            return tuple(bass2jax._bass_exec_p.bind(
                *operands, out_avals=_avals, in_names=_bind, out_names=_outn,
                lowering_input_output_aliases=(),
                sim_require_finite=True, sim_require_nnan=True, nc=_nc,
            ))

        m = dict(params)
        m["x"] = np.ascontiguousarray(x[:t_core])
        dev = jax.devices()[0]
        args = [jax.device_put(np.asarray(m[n]), dev) for n in in_names]
        args += [jax.device_put(z, dev) for z in zero_outs]
        fn = jax.jit(_body)
        outs = fn(*args)
        jax.block_until_ready(outs)
        best = None
        for _ in range(calls):
            t0 = _time.perf_counter()
            outs = fn(*args)
            jax.block_until_ready(outs)
            dt = _time.perf_counter() - t0
            best = dt if best is None else min(best, dt)
        results[r] = best
        print(f"  reps={r}: best call {best*1e3:.3f} ms")
    ns = (results[reps] - results[1]) / (reps - 1) * 1e9
    return ns


def bench(inputs, iters=8):
    """Time `iters` chained kernel executions on HW inside one jit.

    Outputs are fed back as the (normally zero-initialized) output buffers of
    the next iteration; the kernel overwrites every output element, so values
    stay correct and the data dependency serializes executions on-device.
    Returns (ns_per_iter, out_of_last_iter).
    """
    import jax
    from jax.sharding import Mesh, PartitionSpec
    from jax.experimental.shard_map import shard_map
    import time as _time
    from concourse import bass2jax, mybir as _mb

    x = np.asarray(inputs["x_streams"], np.float32).reshape(T, NCF)
    params = prep_params(inputs)
    t_core = T // NCORES
    nc = get_program(t_core)
    bass2jax.install_neuronx_cc_hook()

    partition_name = (
        nc.partition_id_tensor.name if nc.partition_id_tensor else None
    )
    in_names, out_names, out_avals, zero_outs = [], [], [], []
    for alloc in nc.m.functions[0].allocations:
        if not isinstance(alloc, _mb.MemoryLocationSet):
            continue
        name = alloc.memorylocations[0].name
        if alloc.kind == "ExternalInput":
            if name != partition_name:
                in_names.append(name)
        elif alloc.kind == "ExternalOutput":
            out_names.append(name)
            shape = tuple(alloc.tensor_shape)
            dtype = _mb.dt.np(alloc.dtype)
            out_avals.append(jax.core.ShapedArray(shape, dtype))
            zero_outs.append(np.zeros(shape, dtype))
    n_params = len(in_names)

    bind_names = list(in_names) + list(out_names)
    if partition_name is not None:
        bind_names.append(partition_name)

    def body_once(args, outs):
        operands = list(args) + list(outs)
        if partition_name is not None:
            operands.append(bass2jax.partition_id_tensor())
        res = bass2jax._bass_exec_p.bind(
            *operands,
            out_avals=tuple(out_avals),
            in_names=tuple(bind_names),
            out_names=tuple(out_names),
            lowering_input_output_aliases=(),
            sim_require_finite=True,
            sim_require_nnan=True,
            nc=nc,
        )
        return tuple(res)

    def chain(k):
        def _body(*flat):
            args = flat[:n_params]
            outs = flat[n_params:]
            for _ in range(k):
                outs = body_once(args, outs)
            return outs
        return _body

    devices = jax.devices()[:NCORES]
    mesh = Mesh(np.asarray(devices), ("core",))
    in_specs = (PartitionSpec("core"),) * (n_params + len(out_names))
    out_specs = (PartitionSpec("core"),) * len(out_names)

    per_core = []
    for c in range(NCORES):
        m = dict(params)
        m["x"] = np.ascontiguousarray(x[c * t_core:(c + 1) * t_core])
        per_core.append([np.asarray(m[n]) for n in in_names])
    concat_in = [
        np.concatenate([per_core[c][i] for c in range(NCORES)], axis=0)
        for i in range(n_params)
    ]
    concat_zeros = [
        np.zeros((NCORES * z.shape[0], *z.shape[1:]), z.dtype) for z in zero_outs
    ]

    times = {}
    for k in (1, 1 + iters):
        fn = jax.jit(
            shard_map(chain(k), mesh=mesh, in_specs=in_specs,
                      out_specs=out_specs, check_rep=False)
        )
        out_arrs = fn(*concat_in, *concat_zeros)  # compile+warm
        jax.block_until_ready(out_arrs)
        reps = 3
        best = None
        for _ in range(reps):
            t0 = _time.perf_counter()
            out_arrs = fn(*concat_in, *concat_zeros)
            jax.block_until_ready(out_arrs)
            dt = _time.perf_counter() - t0
            best = dt if best is None else min(best, dt)
        times[k] = best
    ns = (times[1 + iters] - times[1]) / iters * 1e9
    out = np.asarray(out_arrs[0]).reshape(NCORES, t_core, NCF).reshape(T, N, C)
    return ns, out



# revision 22
# speedup vs baseline: 1.0791x; 1.0633x over previous
"""MHCLiteBlock Trainium2 kernel.

Data-parallel over T across 8 NeuronCores (1024 tokens/core); all params
replicated. Per core, per 128-token tile:

  1. SWDGE cast-DMA: x fp32 HBM -> xn bf16 SBUF (4 chunks).
  2. ACT Square+accum on xn -> ssq; irms = exp(-0.5*ln(ssq/NC + eps))
     (ln/exp keep every ACT transcendental in ONE table set).
  3. DMA-xbar-transpose xn -> xT chunks [128c, 128t].
  4. proj (PE): proj[t, 32] = sum_k xT_k.T @ wallT_k directly in [t, .] layout.
  5. scaled = (proj * irms) * alpha + bias with alpha/bias negated on cols 0:8;
     eall = Exp(scaled): cols 0:8 = exp(-z) -> sigmoid via DVE 1/(1+u);
     cols 8:32 = softmax numerators. Soft permutation H via perm_aug matmul.
  6. li (DVE): libf = sum_m h_pre_m * x_m, bf16.
  7. M2 (PE): diff = liT.T @ (W_layer.T - I) + b_layer.
  8. Mixing (PE): out_n = sum_m diag(H[n,m]) @ x_m; DVE combine adds
     2*h_post_n * diff and copies PSUM->SBUF; DMA out.

Self-contained: hardcodes shapes; builds the Bass program once and caches it.
"""

import sys

sys.path.insert(0, "/opt/trn_rl_repo")

from contextlib import ExitStack

import ml_dtypes
import numpy as np

import concourse.bass as bass
import concourse.mybir as mybir
import concourse.tile as tile
from concourse import bacc, bass_utils

F32 = mybir.dt.float32
BF16 = mybir.dt.bfloat16
AF = mybir.ActivationFunctionType
ALU = mybir.AluOpType

T, N, C = 8192, 4, 2048
NCF = N * C  # 8192 flattened features
NFACT = 24
NCORES = 8
P = 128  # partitions / tokens per tile
EPS = float(np.finfo(np.float32).eps)


class _OneActSetBacc(bacc.Bacc):
    """Pin every activation to one table set so the per-tile Ln/Exp mix
    doesn't ping-pong ACT_TABLE_LOADs between sets.

    The (name, funcs) list passed to the insertion pass is positional —
    index == act_func_set_id — so entries other than the chosen set are
    emptied (never chosen) while keeping ids intact. All ACT funcs this
    kernel uses (Square, Ln, Exp, Copy) live in natural_log_exp_and_others.
    """

    _ACT_SET = "natural_log_exp_and_others"

    def insert_act_table_loads(self):
        import concourse.mybir as _mb
        from concourse.hw_specs import get_activation_tables
        import bass_rust as _br

        has_activation = any(
            isinstance(i, _mb.InstActivation)
            for b in self.main_func.blocks
            for i in b.instructions
        )
        if not has_activation:
            return
        tables = []
        for name, funcs in get_activation_tables(self.m.arch).items():
            tables.append((name, funcs if name == self._ACT_SET else set()))
        _br.insert_act_table_loads(self, tables)


def build_program(t_core: int, reps: int = 1, num_devices: int = NCORES):
    nt = t_core // P
    nc = _OneActSetBacc(
        "TRN2", target_bir_lowering=False, debug=False, num_devices=num_devices
    )

    x_d = nc.dram_tensor("x", [t_core, NCF], F32, kind="ExternalInput").ap()
    wallt_d = nc.dram_tensor("wallt", [P, 64, 32], BF16, kind="ExternalInput").ap()
    wp_d = nc.dram_tensor("wp", [P, 16, C], BF16, kind="ExternalInput").ap()
    blayer_d = nc.dram_tensor("blayer", [1, C], BF16, kind="ExternalInput").ap()
    perm_d = nc.dram_tensor("permaug", [NFACT, 17], F32, kind="ExternalInput").ap()
    ab_d = nc.dram_tensor("alphab", [2, 32], F32, kind="ExternalInput").ap()
    idbf_d = nc.dram_tensor("idbf", [P, P], BF16, kind="ExternalInput").ap()
    idf32_d = nc.dram_tensor("idf32", [P, P], F32, kind="ExternalInput").ap()
    out_d = nc.dram_tensor("out", [t_core, NCF], F32, kind="ExternalOutput").ap()

    with tile.TileContext(nc) as tc:
        _build_body(
            tc, nt, reps, x_d, wallt_d, wp_d, blayer_d, perm_d, ab_d,
            idbf_d, idf32_d, out_d,
        )
    nc.compile()
    return nc


def _build_body(
    tc, nt, reps, x_d, wallt_d, wp_d, blayer_d, perm_d, ab_d, idbf_d,
    idf32_d, out_d,
):
    nc = tc.nc
    with ExitStack() as ctx:
        singles = ctx.enter_context(tc.tile_pool(name="singles", bufs=1))
        xnp = ctx.enter_context(tc.tile_pool(name="xnp", bufs=2))
        xtp = ctx.enter_context(tc.tile_pool(name="xtp", bufs=6))
        smalls = ctx.enter_context(tc.tile_pool(name="smalls", bufs=3))
        sqp = ctx.enter_context(tc.tile_pool(name="sqp", bufs=1))
        diagp = ctx.enter_context(tc.tile_pool(name="diagp", bufs=2))
        xgp = ctx.enter_context(tc.tile_pool(name="xgp", bufs=3))
        lip = ctx.enter_context(tc.tile_pool(name="lip", bufs=3))
        ps_small = ctx.enter_context(
            tc.tile_pool(name="ps_small", bufs=1, space="PSUM")
        )
        ps_diff = ctx.enter_context(
            tc.tile_pool(name="ps_diff", bufs=2, space="PSUM")
        )
        ps_mix = ctx.enter_context(tc.tile_pool(name="ps_mix", bufs=5, space="PSUM"))

        # ---- small one-time parameter loads ----
        wp_s = singles.tile([P, 16, C], BF16)  # loaded after tile 0's x below
        walls = singles.tile([P, 64, 32], BF16)
        nc.sync.dma_start(out=walls[:], in_=wallt_d[:])
        perm_s = singles.tile([NFACT, 17], F32)
        nc.sync.dma_start(out=perm_s[:], in_=perm_d[:])
        idbf_s = singles.tile([P, P], BF16)
        nc.sync.dma_start(out=idbf_s[:], in_=idbf_d[:])
        idf32_s = singles.tile([P, P], F32)
        nc.sync.dma_start(out=idf32_s[:], in_=idf32_d[:])
        blb = singles.tile([P, C], BF16)
        nc.gpsimd.dma_start(
            out=blb[:],
            in_=bass.AP(tensor=blayer_d.tensor, offset=blayer_d.offset,
                        ap=[[0, P], [1, C]]),
        )
        alpha_b = singles.tile([P, 32], F32)
        nc.gpsimd.dma_start(
            out=alpha_b[:],
            in_=bass.AP(tensor=ab_d.tensor, offset=ab_d.offset,
                        ap=[[0, P], [1, 32]]),
        )
        bias_b = singles.tile([P, 32], F32)
        nc.gpsimd.dma_start(
            out=bias_b[:],
            in_=bass.AP(tensor=ab_d.tensor, offset=ab_d.offset + 32,
                        ap=[[0, P], [1, 32]]),
        )
        eps_t = singles.tile([P, 1], F32)
        nc.vector.memset(eps_t[:], EPS)

        def stage_load(t):
            """x cast-load, sum-of-squares, inv-rms, xbar transposes."""
            rows = slice(t * P, (t + 1) * P)
            st = {}

            ssqp = smalls.tile([P, N], F32, tag="ssqp", name=f"ssqp{t}")
            xn = xnp.tile([P, NCF], BF16, tag="xn", name=f"xn{t}")
            sqt = sqp.tile([P, C], BF16, tag="sqt", name=f"sqt{t}")
            xTs = []
            for m in range(N):
                # fp32 HBM -> bf16 SBUF cast during the DMA (SWDGE)
                nc.gpsimd.dma_start(
                    out=xn[:, m * C:(m + 1) * C],
                    in_=x_d[rows, m * C:(m + 1) * C],
                )
                # only the free-dim accumulator is consumed; sqt is scratch
                nc.scalar.activation(
                    out=sqt[:], in_=xn[:, m * C:(m + 1) * C],
                    func=AF.Square, accum_out=ssqp[:, m:m + 1],
                )
                xT = xtp.tile([P, 16, P], BF16, tag="xT", name=f"xT{t}_{m}")
                nc.sync.dma_start_transpose(
                    out=xT[:], in_=xn[:, m * C:(m + 1) * C]
                )
                xTs.append(xT)

            ssq = smalls.tile([P, 1], F32, tag="ssq", name=f"ssq{t}")
            nc.vector.tensor_reduce(
                out=ssq[:], in_=ssqp[:], axis=mybir.AxisListType.X, op=ALU.add
            )
            # irms = (mean(x^2) + eps)^-0.5 = exp(-0.5 * ln(ssq/NC + eps))
            lssq = smalls.tile([P, 1], F32, tag="lssq", name=f"lssq{t}")
            nc.scalar.activation(
                out=lssq[:], in_=ssq[:], func=AF.Ln, bias=eps_t[:],
                scale=1.0 / NCF,
            )
            irms = smalls.tile([P, 1], F32, tag="irms", name=f"irms{t}")
            nc.scalar.activation(out=irms[:], in_=lssq[:], func=AF.Exp, scale=-0.5)

            st["xn"] = xn
            st["xTs"] = xTs
            st["irms"] = irms
            st["rows"] = rows
            st["t"] = t
            return st

        def stage_coeff(st):
            """Projection, gate coefficients, li + its transpose, diags."""
            xn = st["xn"]
            xTs = st["xTs"]
            irms = st["irms"]
            t = st["t"]

            proj_p = ps_small.tile([P, 32], F32, tag="pssmall", name=f"prp{t}")
            for m in range(N):
                for kk in range(16):
                    k = m * 16 + kk
                    nc.tensor.matmul(
                        proj_p[:], xTs[m][:, kk, :], walls[:, k, :],
                        start=(k == 0), stop=(k == 63),
                    )

            # scaled = (proj * irms) * alpha + bias; alpha/bias negated on 0:8
            scaled = smalls.tile([P, 32], F32, tag="scaled", name=f"scl{t}")
            nc.vector.scalar_tensor_tensor(
                out=scaled[:], in0=proj_p[:], scalar=irms[:], in1=alpha_b[:],
                op0=ALU.mult, op1=ALU.mult,
            )
            nc.vector.tensor_add(scaled[:], scaled[:], bias_b[:])

            # eall: cols 0:8 = exp(-z) (sigmoid input), cols 8:32 = softmax exps
            eall = smalls.tile([P, 32], F32, tag="eall", name=f"eall{t}")
            nc.scalar.activation(out=eall[:], in_=scaled[:], func=AF.Exp)

            # h = 1 / (1 + exp(-z)) for the 8 sigmoid outputs
            hden = smalls.tile([P, 8], F32, tag="hden", name=f"hden{t}")
            nc.vector.tensor_scalar_add(hden[:], eall[:, 0:8], 1.0)
            hps = smalls.tile([P, 8], F32, tag="hps", name=f"hps{t}")
            nc.vector.reciprocal(out=hps[:], in_=hden[:])

            # li early: libf = sum_m h_pre_m * x_m (DVE, bf16), then its
            # xbar transpose fires while the rest of the chain runs.
            libf = lip.tile([P, C], BF16, tag="libf", name=f"libf{t}")
            nc.vector.tensor_scalar_mul(libf[:], xn[:, 0:C], hps[:, 0:1])
            for m in range(1, N):
                nc.vector.scalar_tensor_tensor(
                    out=libf[:], in0=xn[:, m * C:(m + 1) * C],
                    scalar=hps[:, m:m + 1], in1=libf[:],
                    op0=ALU.mult, op1=ALU.add,
                )
            liT = lip.tile([P, 16, P], BF16, tag="liT", name=f"liT{t}")
            nc.sync.dma_start_transpose(out=liT[:], in_=libf[:])

            expsT_p = ps_small.tile([NFACT, P], F32, tag="pssmall", name=f"exT{t}")
            nc.tensor.transpose(expsT_p[:], eall[:, 8:32], idf32_s[:])
            expsT_s = smalls.tile([NFACT, P], F32, tag="expsT_s", name=f"exs{t}")
            nc.scalar.activation(out=expsT_s[:], in_=expsT_p[:], func=AF.Copy)

            haug_p = ps_small.tile([P, 17], F32, tag="pssmall", name=f"hgp{t}")
            nc.tensor.matmul(
                haug_p[:], expsT_s[:], perm_s[:], start=True, stop=True
            )
            hd = smalls.tile([P, 17], F32, tag="hd", name=f"hd{t}")
            nc.scalar.activation(out=hd[:], in_=haug_p[:], func=AF.Copy)

            dinv = smalls.tile([P, 1], F32, tag="dinv", name=f"dinv{t}")
            nc.vector.reciprocal(out=dinv[:], in_=hd[:, 16:17])

            # coeffs cols 0:16 = normalized H (col 4m+n = H[n,m]);
            # 16:20 = 2*h_post
            coeffs = smalls.tile([P, 20], F32, tag="coeffs", name=f"co{t}")
            nc.vector.tensor_scalar_mul(coeffs[:, 0:16], hd[:, 0:16], dinv[:])
            nc.vector.tensor_scalar_mul(coeffs[:, 16:20], hps[:, 4:8], 2.0)

            # diags: j=4m+n -> H[n,m] for the mixing matmuls
            diags = diagp.tile([P, 16, P], BF16, tag="diags", name=f"dg{t}")
            for j in range(16):
                nc.vector.tensor_scalar_mul(
                    diags[:, j, :], idbf_s[:], coeffs[:, j:j + 1]
                )

            st["diags"] = diags
            st["coeffs"] = coeffs
            st["liT"] = liT
            return st

        def stage_b(st):
            """diff = liT.T @ (W.T - I) + b, mixing, store."""
            xn = st["xn"]
            diags = st["diags"]
            rows = st["rows"]
            coeffs = st["coeffs"]
            liT = st["liT"]
            t = st["t"]

            diffbf = lip.tile([P, C], BF16, tag="diffbf", name=f"diffbf{t}")
            for q in range(4):
                cs = slice(q * 512, (q + 1) * 512)
                diff_p = ps_diff.tile([P, 512], F32, tag="diff")
                for k in range(16):
                    nc.tensor.matmul(
                        diff_p[:], liT[:, k, :], wp_s[:, k, cs],
                        start=(k == 0), stop=(k == 15),
                    )
                # diffbf = diff + b_layer (broadcast), cast to bf16
                nc.vector.scalar_tensor_tensor(
                    out=diffbf[:, cs], in0=diff_p[:], scalar=1.0,
                    in1=blb[:, cs], op0=ALU.bypass, op1=ALU.add,
                )

            # ---- mixing: out_n = sum_m diag(H[n,m]) @ x_m + h_post2_n*diff
            for n in range(N):
                outsb = xgp.tile([P, C], F32, tag="outsb", name=f"ou{t}_{n}")
                for cc in range(4):
                    cs = slice(cc * 512, (cc + 1) * 512)
                    mix_p = ps_mix.tile([P, 512], F32, tag="mix",
                                        name=f"mx{t}_{n}_{cc}")
                    for src_ in range(N):
                        nc.tensor.matmul(
                            mix_p[:], diags[:, 4 * src_ + n, :],
                            xn[:, src_ * C + cc * 512: src_ * C + (cc + 1) * 512],
                            start=(src_ == 0), stop=(src_ == 3),
                        )
                    nc.vector.scalar_tensor_tensor(
                        out=outsb[:, cs], in0=diffbf[:, cs],
                        scalar=coeffs[:, 16 + n:17 + n], in1=mix_p[:],
                        op0=ALU.mult, op1=ALU.add,
                    )
                nc.sync.dma_start(
                    out=out_d[rows, n * C:(n + 1) * C], in_=outsb[:]
                )

        # ---- software-pipelined emission ----
        # Per iteration: loads(t+1) first (DMA queues fill early), then the
        # heavy PE work of tile t (diff+mix), then tile t+1's coefficient
        # chain. Keeps ready work at each engine FIFO's head: tile t+1's
        # proj/diags (gated on DMA) never sit ahead of tile t's diff/mix.
        first = True
        pending = None  # tile with coeffs done, stage_b outstanding
        for rep in range(reps):
            for t in range(nt):
                ld = stage_load(t)
                if first:
                    # defer the big weight load until after tile 0's x DMAs
                    nc.sync.dma_start(out=wp_s[:], in_=wp_d[:])
                    first = False
                if pending is not None:
                    stage_b(pending)
                pending = stage_coeff(ld)
        stage_b(pending)


def prep_params(inputs):
    """Host-side parameter preprocessing shared by all cores."""
    bf = ml_dtypes.bfloat16
    W_all = np.asarray(inputs["W_all"], np.float32)
    W_layer = np.asarray(inputs["W_layer"], np.float32)
    b_all = np.asarray(inputs["b_all"], np.float32)
    b_layer = np.asarray(inputs["b_layer"], np.float32)
    perm_mat = np.asarray(inputs["perm_mat"], np.float32)
    a_pre = float(np.asarray(inputs["alpha_pre"]).reshape(-1)[0])
    a_post = float(np.asarray(inputs["alpha_post"]).reshape(-1)[0])
    a_res = float(np.asarray(inputs["alpha_res"]).reshape(-1)[0])

    wallt = np.ascontiguousarray(
        W_all.T.astype(bf).reshape(64, P, 32).transpose(1, 0, 2)
    )
    wp = (np.ascontiguousarray(W_layer.T) - np.eye(C, dtype=np.float32))
    wp = np.ascontiguousarray(wp.astype(bf).reshape(16, P, C).transpose(1, 0, 2))
    blayer = b_layer.astype(bf).reshape(1, C)
    # perm_aug columns in m-major order: col 4m+n = perm_mat[:, n*4+m]; col 16 = 1
    perm_aug = np.zeros((NFACT, 17), np.float32)
    perm_aug[:, :16] = perm_mat.reshape(NFACT, N, N).transpose(0, 2, 1).reshape(
        NFACT, 16
    )
    perm_aug[:, 16] = 1.0
    # cols 0:8 negated: eall = exp(-(alpha*p + b)) there, for sigmoid via 1/(1+u)
    alphab = np.zeros((2, 32), np.float32)
    alphab[0, 0:4] = -a_pre
    alphab[0, 4:8] = -a_post
    alphab[0, 8:32] = a_res
    alphab[1, 0:4] = -b_all[0:4]
    alphab[1, 4:8] = -b_all[4:8]
    alphab[1, 8:32] = b_all[8:32]
    idbf = np.eye(P, dtype=np.float32).astype(bf)
    idf32 = np.eye(P, dtype=np.float32)
    return {
        "wallt": wallt, "wp": wp, "blayer": blayer,
        "permaug": perm_aug, "alphab": alphab, "idbf": idbf, "idf32": idf32,
    }


_PROGRAM_CACHE = {}


def get_program(t_core):
    if t_core not in _PROGRAM_CACHE:
        _PROGRAM_CACHE[t_core] = build_program(t_core)
    return _PROGRAM_CACHE[t_core]


def run(inputs, trace=False):
    x = np.asarray(inputs["x_streams"], np.float32).reshape(T, NCF)
    params = prep_params(inputs)
    t_core = T // NCORES
    nc = get_program(t_core)
    in_maps = []
    for c in range(NCORES):
        m = dict(params)
        m["x"] = np.ascontiguousarray(x[c * t_core:(c + 1) * t_core])
        in_maps.append(m)
    res = bass_utils.run_bass_kernel_spmd(
        nc, in_maps, core_ids=list(range(NCORES)), trace=trace
    )
    out = np.concatenate([r["out"] for r in res.results], axis=0)
    return out.reshape(T, N, C).astype(np.float32), res


def kernel(**inputs) -> np.ndarray:
    out, _ = run(inputs)
    return out


def bench_reps(inputs, reps=5, calls=7):
    """Single-core timing: diff a reps-unrolled program against reps=1.

    Inputs are device-resident; each call is one NEFF execution, so the
    difference isolates (reps-1) kernel-body repetitions.
    """
    import time as _time

    import jax

    from concourse import bass2jax
    from concourse import mybir as _mb

    x = np.asarray(inputs["x_streams"], np.float32).reshape(T, NCF)
    params = prep_params(inputs)
    t_core = T // NCORES
    bass2jax.install_neuronx_cc_hook()

    results = {}
    for r in (1, reps):
        nc = build_program(t_core, reps=r, num_devices=1)
        partition_name = (
            nc.partition_id_tensor.name if nc.partition_id_tensor else None
        )
        in_names, out_names, out_avals, zero_outs = [], [], [], []
        for alloc in nc.m.functions[0].allocations:
            if not isinstance(alloc, _mb.MemoryLocationSet):
                continue
            name = alloc.memorylocations[0].name
            if alloc.kind == "ExternalInput":
                if name != partition_name:
                    in_names.append(name)
            elif alloc.kind == "ExternalOutput":
                out_names.append(name)
                shape = tuple(alloc.tensor_shape)
                dtype = _mb.dt.np(alloc.dtype)
                out_avals.append(jax.core.ShapedArray(shape, dtype))
                zero_outs.append(np.zeros(shape, dtype))
        bind_names = list(in_names) + list(out_names)
        if partition_name is not None:
            bind_names.append(partition_name)

        def _body(*flat, _nc=nc, _bind=tuple(bind_names),
                  _outn=tuple(out_names), _avals=tuple(out_avals),
                  _pn=partition_name):
            operands = list(flat)
            if _pn is not None:
                operands.append(bass2jax.partition_id_tensor())
            return tuple(bass2jax._bass_exec_p.bind(
                *operands, out_avals=_avals, in_names=_bind, out_names=_outn,
                lowering_input_output_aliases=(),
                sim_require_finite=True, sim_require_nnan=True, nc=_nc,
            ))

        m = dict(params)
        m["x"] = np.ascontiguousarray(x[:t_core])
        dev = jax.devices()[0]
        args = [jax.device_put(np.asarray(m[n]), dev) for n in in_names]
        args += [jax.device_put(z, dev) for z in zero_outs]
        fn = jax.jit(_body)
        outs = fn(*args)
        jax.block_until_ready(outs)
        best = None
        for _ in range(calls):
            t0 = _time.perf_counter()
            outs = fn(*args)
            jax.block_until_ready(outs)
            dt = _time.perf_counter() - t0
            best = dt if best is None else min(best, dt)
        results[r] = best
        print(f"  reps={r}: best call {best*1e3:.3f} ms")
    ns = (results[reps] - results[1]) / (reps - 1) * 1e9
    return ns


def bench(inputs, iters=8):
    """Time `iters` chained kernel executions on HW inside one jit.

    Outputs are fed back as the (normally zero-initialized) output buffers of
    the next iteration; the kernel overwrites every output element, so values
    stay correct and the data dependency serializes executions on-device.
    Returns (ns_per_iter, out_of_last_iter).
    """
    import jax
    from jax.sharding import Mesh, PartitionSpec
    from jax.experimental.shard_map import shard_map
    import time as _time
    from concourse import bass2jax, mybir as _mb

    x = np.asarray(inputs["x_streams"], np.float32).reshape(T, NCF)
    params = prep_params(inputs)
    t_core = T // NCORES
    nc = get_program(t_core)
    bass2jax.install_neuronx_cc_hook()

    partition_name = (
        nc.partition_id_tensor.name if nc.partition_id_tensor else None
    )
    in_names, out_names, out_avals, zero_outs = [], [], [], []
    for alloc in nc.m.functions[0].allocations:
        if not isinstance(alloc, _mb.MemoryLocationSet):
            continue
        name = alloc.memorylocations[0].name
        if alloc.kind == "ExternalInput":
            if name != partition_name:
                in_names.append(name)
        elif alloc.kind == "ExternalOutput":
            out_names.append(name)
            shape = tuple(alloc.tensor_shape)
            dtype = _mb.dt.np(alloc.dtype)
            out_avals.append(jax.core.ShapedArray(shape, dtype))
            zero_outs.append(np.zeros(shape, dtype))
    n_params = len(in_names)

    bind_names = list(in_names) + list(out_names)
    if partition_name is not None:
        bind_names.append(partition_name)

    def body_once(args, outs):
        operands = list(args) + list(outs)
        if partition_name is not None:
            operands.append(bass2jax.partition_id_tensor())
        res = bass2jax._bass_exec_p.bind(
            *operands,
            out_avals=tuple(out_avals),
            in_names=tuple(bind_names),
            out_names=tuple(out_names),
            lowering_input_output_aliases=(),
            sim_require_finite=True,
            sim_require_nnan=True,
            nc=nc,
        )
        return tuple(res)

    def chain(k):
        def _body(*flat):
            args = flat[:n_params]
            outs = flat[n_params:]
            for _ in range(k):
                outs = body_once(args, outs)
            return outs
        return _body

    devices = jax.devices()[:NCORES]
    mesh = Mesh(np.asarray(devices), ("core",))
    in_specs = (PartitionSpec("core"),) * (n_params + len(out_names))
    out_specs = (PartitionSpec("core"),) * len(out_names)

    per_core = []
    for c in range(NCORES):
        m = dict(params)
        m["x"] = np.ascontiguousarray(x[c * t_core:(c + 1) * t_core])
        per_core.append([np.asarray(m[n]) for n in in_names])
    concat_in = [
        np.concatenate([per_core[c][i] for c in range(NCORES)], axis=0)
        for i in range(n_params)
    ]
    concat_zeros = [
        np.zeros((NCORES * z.shape[0], *z.shape[1:]), z.dtype) for z in zero_outs
    ]

    times = {}
    for k in (1, 1 + iters):
        fn = jax.jit(
            shard_map(chain(k), mesh=mesh, in_specs=in_specs,
                      out_specs=out_specs, check_rep=False)
        )
        out_arrs = fn(*concat_in, *concat_zeros)  # compile+warm
        jax.block_until_ready(out_arrs)
        reps = 3
        best = None
        for _ in range(reps):
            t0 = _time.perf_counter()
            out_arrs = fn(*concat_in, *concat_zeros)
            jax.block_until_ready(out_arrs)
            dt = _time.perf_counter() - t0
            best = dt if best is None else min(best, dt)
        times[k] = best
    ns = (times[1 + iters] - times[1]) / iters * 1e9
    out = np.asarray(out_arrs[0]).reshape(NCORES, t_core, NCF).reshape(T, N, C)
    return ns, out



# revision 23
# speedup vs baseline: 1.0800x; 1.0009x over previous
"""MHCLiteBlock Trainium2 kernel.

Data-parallel over T across 8 NeuronCores (1024 tokens/core); all params
replicated. Per core, per 128-token tile:

  1. SWDGE cast-DMA: x fp32 HBM -> xn bf16 SBUF (4 chunks).
  2. ACT Square+accum on xn -> ssq; irms = exp(-0.5*ln(ssq/NC + eps))
     (ln/exp keep every ACT transcendental in ONE table set).
  3. DMA-xbar-transpose xn -> xT chunks [128c, 128t].
  4. proj (PE): proj[t, 32] = sum_k xT_k.T @ wallT_k directly in [t, .] layout.
  5. scaled = (proj * irms) * alpha + bias with alpha/bias negated on cols 0:8;
     eall = Exp(scaled): cols 0:8 = exp(-z) -> sigmoid via DVE 1/(1+u);
     cols 8:32 = softmax numerators. Soft permutation H via perm_aug matmul.
  6. li (DVE): libf = sum_m h_pre_m * x_m, bf16.
  7. M2 (PE): diff = liT.T @ (W_layer.T - I) + b_layer.
  8. Mixing (PE): out_n = sum_m diag(H[n,m]) @ x_m; DVE combine adds
     2*h_post_n * diff and copies PSUM->SBUF; DMA out.

Self-contained: hardcodes shapes; builds the Bass program once and caches it.
"""

import sys

sys.path.insert(0, "/opt/trn_rl_repo")

from contextlib import ExitStack

import ml_dtypes
import numpy as np

import concourse.bass as bass
import concourse.mybir as mybir
import concourse.tile as tile
from concourse import bacc, bass_utils

F32 = mybir.dt.float32
BF16 = mybir.dt.bfloat16
AF = mybir.ActivationFunctionType
ALU = mybir.AluOpType

T, N, C = 8192, 4, 2048
NCF = N * C  # 8192 flattened features
NFACT = 24
NCORES = 8
P = 128  # partitions / tokens per tile
EPS = float(np.finfo(np.float32).eps)


class _OneActSetBacc(bacc.Bacc):
    """Pin every activation to one table set so the per-tile Ln/Exp mix
    doesn't ping-pong ACT_TABLE_LOADs between sets.

    The (name, funcs) list passed to the insertion pass is positional —
    index == act_func_set_id — so entries other than the chosen set are
    emptied (never chosen) while keeping ids intact. All ACT funcs this
    kernel uses (Square, Ln, Exp, Copy) live in natural_log_exp_and_others.
    """

    _ACT_SET = "natural_log_exp_and_others"

    def insert_act_table_loads(self):
        import concourse.mybir as _mb
        from concourse.hw_specs import get_activation_tables
        import bass_rust as _br

        has_activation = any(
            isinstance(i, _mb.InstActivation)
            for b in self.main_func.blocks
            for i in b.instructions
        )
        if not has_activation:
            return
        tables = []
        for name, funcs in get_activation_tables(self.m.arch).items():
            tables.append((name, funcs if name == self._ACT_SET else set()))
        _br.insert_act_table_loads(self, tables)


def build_program(t_core: int, reps: int = 1, num_devices: int = NCORES):
    nt = t_core // P
    nc = _OneActSetBacc(
        "TRN2", target_bir_lowering=False, debug=False, num_devices=num_devices
    )

    x_d = nc.dram_tensor("x", [t_core, NCF], F32, kind="ExternalInput").ap()
    wallt_d = nc.dram_tensor("wallt", [P, 64, 32], BF16, kind="ExternalInput").ap()
    wp_d = nc.dram_tensor("wp", [P, 16, C], BF16, kind="ExternalInput").ap()
    blayer_d = nc.dram_tensor("blayer", [1, C], BF16, kind="ExternalInput").ap()
    perm_d = nc.dram_tensor("permaug", [NFACT, 17], F32, kind="ExternalInput").ap()
    ab_d = nc.dram_tensor("alphab", [2, 32], F32, kind="ExternalInput").ap()
    idbf_d = nc.dram_tensor("idbf", [P, P], BF16, kind="ExternalInput").ap()
    idf32_d = nc.dram_tensor("idf32", [P, P], F32, kind="ExternalInput").ap()
    out_d = nc.dram_tensor("out", [t_core, NCF], F32, kind="ExternalOutput").ap()

    with tile.TileContext(nc) as tc:
        _build_body(
            tc, nt, reps, x_d, wallt_d, wp_d, blayer_d, perm_d, ab_d,
            idbf_d, idf32_d, out_d,
        )
    nc.compile()
    return nc


def _build_body(
    tc, nt, reps, x_d, wallt_d, wp_d, blayer_d, perm_d, ab_d, idbf_d,
    idf32_d, out_d,
):
    nc = tc.nc
    with ExitStack() as ctx:
        singles = ctx.enter_context(tc.tile_pool(name="singles", bufs=1))
        xnp = ctx.enter_context(tc.tile_pool(name="xnp", bufs=2))
        xtp = ctx.enter_context(tc.tile_pool(name="xtp", bufs=6))
        smalls = ctx.enter_context(tc.tile_pool(name="smalls", bufs=3))
        sqp = ctx.enter_context(tc.tile_pool(name="sqp", bufs=1))
        diagp = ctx.enter_context(tc.tile_pool(name="diagp", bufs=2))
        xgp = ctx.enter_context(tc.tile_pool(name="xgp", bufs=3))
        lip = ctx.enter_context(tc.tile_pool(name="lip", bufs=3))
        ps_small = ctx.enter_context(
            tc.tile_pool(name="ps_small", bufs=1, space="PSUM")
        )
        ps_diff = ctx.enter_context(
            tc.tile_pool(name="ps_diff", bufs=2, space="PSUM")
        )
        ps_mix = ctx.enter_context(tc.tile_pool(name="ps_mix", bufs=5, space="PSUM"))

        # ---- small one-time parameter loads ----
        wp_s = singles.tile([P, 16, C], BF16)  # loaded after tile 0's x below
        walls = singles.tile([P, 64, 32], BF16)
        nc.sync.dma_start(out=walls[:], in_=wallt_d[:])
        perm_s = singles.tile([NFACT, 17], F32)
        nc.sync.dma_start(out=perm_s[:], in_=perm_d[:])
        idbf_s = singles.tile([P, P], BF16)
        nc.sync.dma_start(out=idbf_s[:], in_=idbf_d[:])
        idf32_s = singles.tile([P, P], F32)
        nc.sync.dma_start(out=idf32_s[:], in_=idf32_d[:])
        blb = singles.tile([P, C], BF16)
        nc.gpsimd.dma_start(
            out=blb[:],
            in_=bass.AP(tensor=blayer_d.tensor, offset=blayer_d.offset,
                        ap=[[0, P], [1, C]]),
        )
        alpha_b = singles.tile([P, 32], F32)
        nc.gpsimd.dma_start(
            out=alpha_b[:],
            in_=bass.AP(tensor=ab_d.tensor, offset=ab_d.offset,
                        ap=[[0, P], [1, 32]]),
        )
        bias_b = singles.tile([P, 32], F32)
        nc.gpsimd.dma_start(
            out=bias_b[:],
            in_=bass.AP(tensor=ab_d.tensor, offset=ab_d.offset + 32,
                        ap=[[0, P], [1, 32]]),
        )
        eps_t = singles.tile([P, 1], F32)
        nc.vector.memset(eps_t[:], EPS)

        def stage_load(t):
            """x cast-load, sum-of-squares, inv-rms, xbar transposes."""
            rows = slice(t * P, (t + 1) * P)
            st = {}

            ssqp = smalls.tile([P, N], F32, tag="ssqp", name=f"ssqp{t}")
            xn = xnp.tile([P, NCF], BF16, tag="xn", name=f"xn{t}")
            sqt = sqp.tile([P, C], BF16, tag="sqt", name=f"sqt{t}")
            xTs = []
            for m in range(N):
                # fp32 HBM -> bf16 SBUF cast during the DMA (SWDGE)
                nc.gpsimd.dma_start(
                    out=xn[:, m * C:(m + 1) * C],
                    in_=x_d[rows, m * C:(m + 1) * C],
                )
                # only the free-dim accumulator is consumed; sqt is scratch
                nc.scalar.activation(
                    out=sqt[:], in_=xn[:, m * C:(m + 1) * C],
                    func=AF.Square, accum_out=ssqp[:, m:m + 1],
                )
                xT = xtp.tile([P, 16, P], BF16, tag="xT", name=f"xT{t}_{m}")
                nc.sync.dma_start_transpose(
                    out=xT[:], in_=xn[:, m * C:(m + 1) * C]
                )
                xTs.append(xT)

            ssq = smalls.tile([P, 1], F32, tag="ssq", name=f"ssq{t}")
            nc.vector.tensor_reduce(
                out=ssq[:], in_=ssqp[:], axis=mybir.AxisListType.X, op=ALU.add
            )
            # irms = (mean(x^2) + eps)^-0.5 = exp(-0.5 * ln(ssq/NC + eps))
            lssq = smalls.tile([P, 1], F32, tag="lssq", name=f"lssq{t}")
            nc.scalar.activation(
                out=lssq[:], in_=ssq[:], func=AF.Ln, bias=eps_t[:],
                scale=1.0 / NCF,
            )
            irms = smalls.tile([P, 1], F32, tag="irms", name=f"irms{t}")
            nc.scalar.activation(out=irms[:], in_=lssq[:], func=AF.Exp, scale=-0.5)

            st["xn"] = xn
            st["xTs"] = xTs
            st["irms"] = irms
            st["rows"] = rows
            st["t"] = t
            return st

        def stage_coeff(st):
            """Projection, gate coefficients, li + its transpose, diags."""
            xn = st["xn"]
            xTs = st["xTs"]
            irms = st["irms"]
            t = st["t"]

            proj_p = ps_small.tile([P, 32], F32, tag="pssmall", name=f"prp{t}")
            for m in range(N):
                for kk in range(16):
                    k = m * 16 + kk
                    nc.tensor.matmul(
                        proj_p[:], xTs[m][:, kk, :], walls[:, k, :],
                        start=(k == 0), stop=(k == 63),
                    )

            # scaled = (proj * irms) * alpha + bias; alpha/bias negated on 0:8
            scaled = smalls.tile([P, 32], F32, tag="scaled", name=f"scl{t}")
            nc.vector.scalar_tensor_tensor(
                out=scaled[:], in0=proj_p[:], scalar=irms[:], in1=alpha_b[:],
                op0=ALU.mult, op1=ALU.mult,
            )
            nc.vector.tensor_add(scaled[:], scaled[:], bias_b[:])

            # eall: cols 0:8 = exp(-z) (sigmoid input), cols 8:32 = softmax exps
            eall = smalls.tile([P, 32], F32, tag="eall", name=f"eall{t}")
            nc.scalar.activation(out=eall[:], in_=scaled[:], func=AF.Exp)

            # h = 1 / (1 + exp(-z)) for the 8 sigmoid outputs
            hden = smalls.tile([P, 8], F32, tag="hden", name=f"hden{t}")
            nc.vector.tensor_scalar_add(hden[:], eall[:, 0:8], 1.0)
            hps = smalls.tile([P, 8], F32, tag="hps", name=f"hps{t}")
            nc.vector.reciprocal(out=hps[:], in_=hden[:])

            # li early: libf = sum_m h_pre_m * x_m (DVE, bf16), then its
            # xbar transpose fires while the rest of the chain runs.
            libf = lip.tile([P, C], BF16, tag="libf", name=f"libf{t}")
            nc.vector.tensor_scalar_mul(libf[:], xn[:, 0:C], hps[:, 0:1])
            for m in range(1, N):
                nc.vector.scalar_tensor_tensor(
                    out=libf[:], in0=xn[:, m * C:(m + 1) * C],
                    scalar=hps[:, m:m + 1], in1=libf[:],
                    op0=ALU.mult, op1=ALU.add,
                )
            liT = lip.tile([P, 16, P], BF16, tag="liT", name=f"liT{t}")
            nc.sync.dma_start_transpose(out=liT[:], in_=libf[:])

            expsT_p = ps_small.tile([NFACT, P], F32, tag="pssmall", name=f"exT{t}")
            nc.tensor.transpose(expsT_p[:], eall[:, 8:32], idf32_s[:])
            expsT_s = smalls.tile([NFACT, P], F32, tag="expsT_s", name=f"exs{t}")
            nc.scalar.activation(out=expsT_s[:], in_=expsT_p[:], func=AF.Copy)

            haug_p = ps_small.tile([P, 17], F32, tag="pssmall", name=f"hgp{t}")
            nc.tensor.matmul(
                haug_p[:], expsT_s[:], perm_s[:], start=True, stop=True
            )
            # normalize straight out of PSUM: drops an ACT copy plus two
            # cross-engine hops from the per-beat critical chain
            dinv = smalls.tile([P, 1], F32, tag="dinv", name=f"dinv{t}")
            nc.vector.reciprocal(out=dinv[:], in_=haug_p[:, 16:17])

            # coeffs cols 0:16 = normalized H (col 4m+n = H[n,m]);
            # 16:20 = 2*h_post
            coeffs = smalls.tile([P, 20], F32, tag="coeffs", name=f"co{t}")
            nc.vector.tensor_scalar_mul(
                coeffs[:, 0:16], haug_p[:, 0:16], dinv[:]
            )
            nc.vector.tensor_scalar_mul(coeffs[:, 16:20], hps[:, 4:8], 2.0)

            # diags: j=4m+n -> H[n,m] for the mixing matmuls
            diags = diagp.tile([P, 16, P], BF16, tag="diags", name=f"dg{t}")
            for j in range(16):
                nc.vector.tensor_scalar_mul(
                    diags[:, j, :], idbf_s[:], coeffs[:, j:j + 1]
                )

            st["diags"] = diags
            st["coeffs"] = coeffs
            st["liT"] = liT
            return st

        def stage_b(st):
            """diff = liT.T @ (W.T - I) + b, mixing, store."""
            xn = st["xn"]
            diags = st["diags"]
            rows = st["rows"]
            coeffs = st["coeffs"]
            liT = st["liT"]
            t = st["t"]

            diffbf = lip.tile([P, C], BF16, tag="diffbf", name=f"diffbf{t}")
            for q in range(4):
                cs = slice(q * 512, (q + 1) * 512)
                diff_p = ps_diff.tile([P, 512], F32, tag="diff")
                for k in range(16):
                    nc.tensor.matmul(
                        diff_p[:], liT[:, k, :], wp_s[:, k, cs],
                        start=(k == 0), stop=(k == 15),
                    )
                # diffbf = diff + b_layer (broadcast), cast to bf16
                nc.vector.scalar_tensor_tensor(
                    out=diffbf[:, cs], in0=diff_p[:], scalar=1.0,
                    in1=blb[:, cs], op0=ALU.bypass, op1=ALU.add,
                )

            # ---- mixing: out_n = sum_m diag(H[n,m]) @ x_m + h_post2_n*diff
            for n in range(N):
                outsb = xgp.tile([P, C], F32, tag="outsb", name=f"ou{t}_{n}")
                for cc in range(4):
                    cs = slice(cc * 512, (cc + 1) * 512)
                    mix_p = ps_mix.tile([P, 512], F32, tag="mix",
                                        name=f"mx{t}_{n}_{cc}")
                    for src_ in range(N):
                        nc.tensor.matmul(
                            mix_p[:], diags[:, 4 * src_ + n, :],
                            xn[:, src_ * C + cc * 512: src_ * C + (cc + 1) * 512],
                            start=(src_ == 0), stop=(src_ == 3),
                        )
                    nc.vector.scalar_tensor_tensor(
                        out=outsb[:, cs], in0=diffbf[:, cs],
                        scalar=coeffs[:, 16 + n:17 + n], in1=mix_p[:],
                        op0=ALU.mult, op1=ALU.add,
                    )
                nc.sync.dma_start(
                    out=out_d[rows, n * C:(n + 1) * C], in_=outsb[:]
                )

        # ---- software-pipelined emission ----
        # Per iteration: loads(t+1) first (DMA queues fill early), then the
        # heavy PE work of tile t (diff+mix), then tile t+1's coefficient
        # chain. Keeps ready work at each engine FIFO's head: tile t+1's
        # proj/diags (gated on DMA) never sit ahead of tile t's diff/mix.
        first = True
        pending = None  # tile with coeffs done, stage_b outstanding
        for rep in range(reps):
            for t in range(nt):
                ld = stage_load(t)
                if first:
                    # defer the big weight load until after tile 0's x DMAs
                    nc.sync.dma_start(out=wp_s[:], in_=wp_d[:])
                    first = False
                if pending is not None:
                    stage_b(pending)
                pending = stage_coeff(ld)
        stage_b(pending)


def prep_params(inputs):
    """Host-side parameter preprocessing shared by all cores."""
    bf = ml_dtypes.bfloat16
    W_all = np.asarray(inputs["W_all"], np.float32)
    W_layer = np.asarray(inputs["W_layer"], np.float32)
    b_all = np.asarray(inputs["b_all"], np.float32)
    b_layer = np.asarray(inputs["b_layer"], np.float32)
    perm_mat = np.asarray(inputs["perm_mat"], np.float32)
    a_pre = float(np.asarray(inputs["alpha_pre"]).reshape(-1)[0])
    a_post = float(np.asarray(inputs["alpha_post"]).reshape(-1)[0])
    a_res = float(np.asarray(inputs["alpha_res"]).reshape(-1)[0])

    wallt = np.ascontiguousarray(
        W_all.T.astype(bf).reshape(64, P, 32).transpose(1, 0, 2)
    )
    wp = (np.ascontiguousarray(W_layer.T) - np.eye(C, dtype=np.float32))
    wp = np.ascontiguousarray(wp.astype(bf).reshape(16, P, C).transpose(1, 0, 2))
    blayer = b_layer.astype(bf).reshape(1, C)
    # perm_aug columns in m-major order: col 4m+n = perm_mat[:, n*4+m]; col 16 = 1
    perm_aug = np.zeros((NFACT, 17), np.float32)
    perm_aug[:, :16] = perm_mat.reshape(NFACT, N, N).transpose(0, 2, 1).reshape(
        NFACT, 16
    )
    perm_aug[:, 16] = 1.0
    # cols 0:8 negated: eall = exp(-(alpha*p + b)) there, for sigmoid via 1/(1+u)
    alphab = np.zeros((2, 32), np.float32)
    alphab[0, 0:4] = -a_pre
    alphab[0, 4:8] = -a_post
    alphab[0, 8:32] = a_res
    alphab[1, 0:4] = -b_all[0:4]
    alphab[1, 4:8] = -b_all[4:8]
    alphab[1, 8:32] = b_all[8:32]
    idbf = np.eye(P, dtype=np.float32).astype(bf)
    idf32 = np.eye(P, dtype=np.float32)
    return {
        "wallt": wallt, "wp": wp, "blayer": blayer,
        "permaug": perm_aug, "alphab": alphab, "idbf": idbf, "idf32": idf32,
    }


_PROGRAM_CACHE = {}


def get_program(t_core):
    if t_core not in _PROGRAM_CACHE:
        _PROGRAM_CACHE[t_core] = build_program(t_core)
    return _PROGRAM_CACHE[t_core]


def run(inputs, trace=False):
    x = np.asarray(inputs["x_streams"], np.float32).reshape(T, NCF)
    params = prep_params(inputs)
    t_core = T // NCORES
    nc = get_program(t_core)
    in_maps = []
    for c in range(NCORES):
        m = dict(params)
        m["x"] = np.ascontiguousarray(x[c * t_core:(c + 1) * t_core])
        in_maps.append(m)
    res = bass_utils.run_bass_kernel_spmd(
        nc, in_maps, core_ids=list(range(NCORES)), trace=trace
    )
    out = np.concatenate([r["out"] for r in res.results], axis=0)
    return out.reshape(T, N, C).astype(np.float32), res


def kernel(**inputs) -> np.ndarray:
    out, _ = run(inputs)
    return out


def bench_reps(inputs, reps=5, calls=7):
    """Single-core timing: diff a reps-unrolled program against reps=1.

    Inputs are device-resident; each call is one NEFF execution, so the
    difference isolates (reps-1) kernel-body repetitions.
    """
    import time as _time

    import jax

    from concourse import bass2jax
    from concourse import mybir as _mb

    x = np.asarray(inputs["x_streams"], np.float32).reshape(T, NCF)
    params = prep_params(inputs)
    t_core = T // NCORES
    bass2jax.install_neuronx_cc_hook()

    results = {}
    for r in (1, reps):
        nc = build_program(t_core, reps=r, num_devices=1)
        partition_name = (
            nc.partition_id_tensor.name if nc.partition_id_tensor else None
        )
        in_names, out_names, out_avals, zero_outs = [], [], [], []
        for alloc in nc.m.functions[0].allocations:
            if not isinstance(alloc, _mb.MemoryLocationSet):
                continue
            name = alloc.memorylocations[0].name
            if alloc.kind == "ExternalInput":
                if name != partition_name:
                    in_names.append(name)
            elif alloc.kind == "ExternalOutput":
                out_names.append(name)
                shape = tuple(alloc.tensor_shape)
                dtype = _mb.dt.np(alloc.dtype)
                out_avals.append(jax.core.ShapedArray(shape, dtype))
                zero_outs.append(np.zeros(shape, dtype))
        bind_names = list(in_names) + list(out_names)
        if partition_name is not None:
            bind_names.append(partition_name)

        def _body(*flat, _nc=nc, _bind=tuple(bind_names),
                  _outn=tuple(out_names), _avals=tuple(out_avals),
                  _pn=partition_name):
            operands = list(flat)
            if _pn is not None:
                operands.append(bass2jax.partition_id_tensor())
            return tuple(bass2jax._bass_exec_p.bind(
                *operands, out_avals=_avals, in_names=_bind, out_names=_outn,
                lowering_input_output_aliases=(),
                sim_require_finite=True, sim_require_nnan=True, nc=_nc,
            ))

        m = dict(params)
        m["x"] = np.ascontiguousarray(x[:t_core])
        dev = jax.devices()[0]
        args = [jax.device_put(np.asarray(m[n]), dev) for n in in_names]
        args += [jax.device_put(z, dev) for z in zero_outs]
        fn = jax.jit(_body)
        outs = fn(*args)
        jax.block_until_ready(outs)
        best = None
        for _ in range(calls):
            t0 = _time.perf_counter()
            outs = fn(*args)
            jax.block_until_ready(outs)
            dt = _time.perf_counter() - t0
            best = dt if best is None else min(best, dt)
        results[r] = best
        print(f"  reps={r}: best call {best*1e3:.3f} ms")
    ns = (results[reps] - results[1]) / (reps - 1) * 1e9
    return ns


def bench(inputs, iters=8):
    """Time `iters` chained kernel executions on HW inside one jit.

    Outputs are fed back as the (normally zero-initialized) output buffers of
    the next iteration; the kernel overwrites every output element, so values
    stay correct and the data dependency serializes executions on-device.
    Returns (ns_per_iter, out_of_last_iter).
    """
    import jax
    from jax.sharding import Mesh, PartitionSpec
    from jax.experimental.shard_map import shard_map
    import time as _time
    from concourse import bass2jax, mybir as _mb

    x = np.asarray(inputs["x_streams"], np.float32).reshape(T, NCF)
    params = prep_params(inputs)
    t_core = T // NCORES
    nc = get_program(t_core)
    bass2jax.install_neuronx_cc_hook()

    partition_name = (
        nc.partition_id_tensor.name if nc.partition_id_tensor else None
    )
    in_names, out_names, out_avals, zero_outs = [], [], [], []
    for alloc in nc.m.functions[0].allocations:
        if not isinstance(alloc, _mb.MemoryLocationSet):
            continue
        name = alloc.memorylocations[0].name
        if alloc.kind == "ExternalInput":
            if name != partition_name:
                in_names.append(name)
        elif alloc.kind == "ExternalOutput":
            out_names.append(name)
            shape = tuple(alloc.tensor_shape)
            dtype = _mb.dt.np(alloc.dtype)
            out_avals.append(jax.core.ShapedArray(shape, dtype))
            zero_outs.append(np.zeros(shape, dtype))
    n_params = len(in_names)

    bind_names = list(in_names) + list(out_names)
    if partition_name is not None:
        bind_names.append(partition_name)

    def body_once(args, outs):
        operands = list(args) + list(outs)
        if partition_name is not None:
            operands.append(bass2jax.partition_id_tensor())
        res = bass2jax._bass_exec_p.bind(
            *operands,
            out_avals=tuple(out_avals),
            in_names=tuple(bind_names),
            out_names=tuple(out_names),
            lowering_input_output_aliases=(),
            sim_require_finite=True,
            sim_require_nnan=True,
            nc=nc,
        )
        return tuple(res)

    def chain(k):
        def _body(*flat):
            args = flat[:n_params]
            outs = flat[n_params:]
            for _ in range(k):
                outs = body_once(args, outs)
            return outs
        return _body

    devices = jax.devices()[:NCORES]
    mesh = Mesh(np.asarray(devices), ("core",))
    in_specs = (PartitionSpec("core"),) * (n_params + len(out_names))
    out_specs = (PartitionSpec("core"),) * len(out_names)

    per_core = []
    for c in range(NCORES):
        m = dict(params)
        m["x"] = np.ascontiguousarray(x[c * t_core:(c + 1) * t_core])
        per_core.append([np.asarray(m[n]) for n in in_names])
    concat_in = [
        np.concatenate([per_core[c][i] for c in range(NCORES)], axis=0)
        for i in range(n_params)
    ]
    concat_zeros = [
        np.zeros((NCORES * z.shape[0], *z.shape[1:]), z.dtype) for z in zero_outs
    ]

    times = {}
    for k in (1, 1 + iters):
        fn = jax.jit(
            shard_map(chain(k), mesh=mesh, in_specs=in_specs,
                      out_specs=out_specs, check_rep=False)
        )
        out_arrs = fn(*concat_in, *concat_zeros)  # compile+warm
        jax.block_until_ready(out_arrs)
        reps = 3
        best = None
        for _ in range(reps):
            t0 = _time.perf_counter()
            out_arrs = fn(*concat_in, *concat_zeros)
            jax.block_until_ready(out_arrs)
            dt = _time.perf_counter() - t0
            best = dt if best is None else min(best, dt)
        times[k] = best
    ns = (times[1 + iters] - times[1]) / iters * 1e9
    out = np.asarray(out_arrs[0]).reshape(NCORES, t_core, NCF).reshape(T, N, C)
    return ns, out



# revision 24
# speedup vs baseline: 1.0819x; 1.0018x over previous
"""MHCLiteBlock Trainium2 kernel.

Data-parallel over T across 8 NeuronCores (1024 tokens/core); all params
replicated. Per core, per 128-token tile:

  1. SWDGE cast-DMA: x fp32 HBM -> xn bf16 SBUF (4 chunks).
  2. ACT Square+accum on xn -> ssq; irms = exp(-0.5*ln(ssq/NC + eps))
     (ln/exp keep every ACT transcendental in ONE table set).
  3. DMA-xbar-transpose xn -> xT chunks [128c, 128t].
  4. proj (PE): proj[t, 32] = sum_k xT_k.T @ wallT_k directly in [t, .] layout.
  5. scaled = (proj * irms) * alpha + bias with alpha/bias negated on cols 0:8;
     eall = Exp(scaled): cols 0:8 = exp(-z) -> sigmoid via DVE 1/(1+u);
     cols 8:32 = softmax numerators. Soft permutation H via perm_aug matmul.
  6. li (DVE): libf = sum_m h_pre_m * x_m, bf16.
  7. M2 (PE): diff = liT.T @ (W_layer.T - I) + b_layer.
  8. Mixing (PE): out_n = sum_m diag(H[n,m]) @ x_m; DVE combine adds
     2*h_post_n * diff and copies PSUM->SBUF; DMA out.

Self-contained: hardcodes shapes; builds the Bass program once and caches it.
"""

import sys

sys.path.insert(0, "/opt/trn_rl_repo")

from contextlib import ExitStack

import ml_dtypes
import numpy as np

import concourse.bass as bass
import concourse.mybir as mybir
import concourse.tile as tile
from concourse import bacc, bass_utils

F32 = mybir.dt.float32
BF16 = mybir.dt.bfloat16
AF = mybir.ActivationFunctionType
ALU = mybir.AluOpType

T, N, C = 8192, 4, 2048
NCF = N * C  # 8192 flattened features
NFACT = 24
NCORES = 8
P = 128  # partitions / tokens per tile
EPS = float(np.finfo(np.float32).eps)


class _OneActSetBacc(bacc.Bacc):
    """Pin every activation to one table set so the per-tile Ln/Exp mix
    doesn't ping-pong ACT_TABLE_LOADs between sets.

    The (name, funcs) list passed to the insertion pass is positional —
    index == act_func_set_id — so entries other than the chosen set are
    emptied (never chosen) while keeping ids intact. All ACT funcs this
    kernel uses (Square, Ln, Exp, Copy) live in natural_log_exp_and_others.
    """

    _ACT_SET = "natural_log_exp_and_others"

    def insert_act_table_loads(self):
        import concourse.mybir as _mb
        from concourse.hw_specs import get_activation_tables
        import bass_rust as _br

        has_activation = any(
            isinstance(i, _mb.InstActivation)
            for b in self.main_func.blocks
            for i in b.instructions
        )
        if not has_activation:
            return
        tables = []
        for name, funcs in get_activation_tables(self.m.arch).items():
            tables.append((name, funcs if name == self._ACT_SET else set()))
        _br.insert_act_table_loads(self, tables)


def build_program(t_core: int, reps: int = 1, num_devices: int = NCORES):
    nt = t_core // P
    nc = _OneActSetBacc(
        "TRN2", target_bir_lowering=False, debug=False, num_devices=num_devices
    )

    x_d = nc.dram_tensor("x", [t_core, NCF], F32, kind="ExternalInput").ap()
    wallt_d = nc.dram_tensor("wallt", [P, 64, 32], BF16, kind="ExternalInput").ap()
    wp_d = nc.dram_tensor("wp", [P, 16, C], BF16, kind="ExternalInput").ap()
    blayer_d = nc.dram_tensor("blayer", [1, C], BF16, kind="ExternalInput").ap()
    perm_d = nc.dram_tensor("permaug", [NFACT, 17], F32, kind="ExternalInput").ap()
    ab_d = nc.dram_tensor("alphab", [2, 32], F32, kind="ExternalInput").ap()
    idbf_d = nc.dram_tensor("idbf", [P, P], BF16, kind="ExternalInput").ap()
    idf32_d = nc.dram_tensor("idf32", [P, P], F32, kind="ExternalInput").ap()
    out_d = nc.dram_tensor("out", [t_core, NCF], F32, kind="ExternalOutput").ap()

    with tile.TileContext(nc) as tc:
        _build_body(
            tc, nt, reps, x_d, wallt_d, wp_d, blayer_d, perm_d, ab_d,
            idbf_d, idf32_d, out_d,
        )
    nc.compile()
    return nc


def _build_body(
    tc, nt, reps, x_d, wallt_d, wp_d, blayer_d, perm_d, ab_d, idbf_d,
    idf32_d, out_d,
):
    nc = tc.nc
    with ExitStack() as ctx:
        singles = ctx.enter_context(tc.tile_pool(name="singles", bufs=1))
        xnp = ctx.enter_context(tc.tile_pool(name="xnp", bufs=2))
        xtp = ctx.enter_context(tc.tile_pool(name="xtp", bufs=6))
        smalls = ctx.enter_context(tc.tile_pool(name="smalls", bufs=3))
        sqp = ctx.enter_context(tc.tile_pool(name="sqp", bufs=1))
        diagp = ctx.enter_context(tc.tile_pool(name="diagp", bufs=2))
        xgp = ctx.enter_context(tc.tile_pool(name="xgp", bufs=3))
        lip = ctx.enter_context(tc.tile_pool(name="lip", bufs=3))
        ps_small = ctx.enter_context(
            tc.tile_pool(name="ps_small", bufs=1, space="PSUM")
        )
        ps_diff = ctx.enter_context(
            tc.tile_pool(name="ps_diff", bufs=2, space="PSUM")
        )
        ps_mix = ctx.enter_context(tc.tile_pool(name="ps_mix", bufs=5, space="PSUM"))

        # ---- small one-time parameter loads ----
        wp_s = singles.tile([P, 16, C], BF16)  # loaded after tile 0's x below
        walls = singles.tile([P, 64, 32], BF16)
        nc.sync.dma_start(out=walls[:], in_=wallt_d[:])
        perm_s = singles.tile([NFACT, 17], F32)
        nc.sync.dma_start(out=perm_s[:], in_=perm_d[:])
        idbf_s = singles.tile([P, P], BF16)
        nc.sync.dma_start(out=idbf_s[:], in_=idbf_d[:])
        idf32_s = singles.tile([P, P], F32)
        nc.sync.dma_start(out=idf32_s[:], in_=idf32_d[:])
        blb = singles.tile([P, C], BF16)
        nc.gpsimd.dma_start(
            out=blb[:],
            in_=bass.AP(tensor=blayer_d.tensor, offset=blayer_d.offset,
                        ap=[[0, P], [1, C]]),
        )
        alpha_b = singles.tile([P, 32], F32)
        nc.gpsimd.dma_start(
            out=alpha_b[:],
            in_=bass.AP(tensor=ab_d.tensor, offset=ab_d.offset,
                        ap=[[0, P], [1, 32]]),
        )
        bias_b = singles.tile([P, 32], F32)
        nc.gpsimd.dma_start(
            out=bias_b[:],
            in_=bass.AP(tensor=ab_d.tensor, offset=ab_d.offset + 32,
                        ap=[[0, P], [1, 32]]),
        )
        eps_t = singles.tile([P, 1], F32)
        nc.vector.memset(eps_t[:], EPS)

        def stage_load(t):
            """x cast-load, sum-of-squares, inv-rms, xbar transposes."""
            rows = slice(t * P, (t + 1) * P)
            st = {}

            ssqp = smalls.tile([P, N], F32, tag="ssqp", name=f"ssqp{t}")
            xn = xnp.tile([P, NCF], BF16, tag="xn", name=f"xn{t}")
            sqt = sqp.tile([P, C], BF16, tag="sqt", name=f"sqt{t}")
            xTs = []
            for m in range(N):
                # fp32 HBM -> bf16 SBUF cast during the DMA (SWDGE)
                nc.gpsimd.dma_start(
                    out=xn[:, m * C:(m + 1) * C],
                    in_=x_d[rows, m * C:(m + 1) * C],
                )
                # only the free-dim accumulator is consumed; sqt is scratch
                nc.scalar.activation(
                    out=sqt[:], in_=xn[:, m * C:(m + 1) * C],
                    func=AF.Square, accum_out=ssqp[:, m:m + 1],
                )
                xT = xtp.tile([P, 16, P], BF16, tag="xT", name=f"xT{t}_{m}")
                nc.sync.dma_start_transpose(
                    out=xT[:], in_=xn[:, m * C:(m + 1) * C]
                )
                xTs.append(xT)

            ssq = smalls.tile([P, 1], F32, tag="ssq", name=f"ssq{t}")
            nc.vector.tensor_reduce(
                out=ssq[:], in_=ssqp[:], axis=mybir.AxisListType.X, op=ALU.add
            )
            # irms = (mean(x^2) + eps)^-0.5 = exp(-0.5 * ln(ssq/NC + eps))
            lssq = smalls.tile([P, 1], F32, tag="lssq", name=f"lssq{t}")
            nc.scalar.activation(
                out=lssq[:], in_=ssq[:], func=AF.Ln, bias=eps_t[:],
                scale=1.0 / NCF,
            )
            irms = smalls.tile([P, 1], F32, tag="irms", name=f"irms{t}")
            nc.scalar.activation(out=irms[:], in_=lssq[:], func=AF.Exp, scale=-0.5)

            st["xn"] = xn
            st["xTs"] = xTs
            st["irms"] = irms
            st["rows"] = rows
            st["t"] = t
            return st

        def stage_coeff(st):
            """Projection, gate coefficients, li + its transpose, diags."""
            xn = st["xn"]
            xTs = st["xTs"]
            irms = st["irms"]
            t = st["t"]

            proj_p = ps_small.tile([P, 32], F32, tag="pssmall", name=f"prp{t}")
            for m in range(N):
                for kk in range(16):
                    k = m * 16 + kk
                    nc.tensor.matmul(
                        proj_p[:], xTs[m][:, kk, :], walls[:, k, :],
                        start=(k == 0), stop=(k == 63),
                    )

            # scaled = (proj * irms) * alpha + bias; alpha/bias negated on 0:8
            scaled = smalls.tile([P, 32], F32, tag="scaled", name=f"scl{t}")
            nc.vector.scalar_tensor_tensor(
                out=scaled[:], in0=proj_p[:], scalar=irms[:], in1=alpha_b[:],
                op0=ALU.mult, op1=ALU.mult,
            )
            nc.vector.tensor_add(scaled[:], scaled[:], bias_b[:])

            # eall: cols 0:8 = exp(-z) (sigmoid input), cols 8:32 = softmax exps
            eall = smalls.tile([P, 32], F32, tag="eall", name=f"eall{t}")
            nc.scalar.activation(out=eall[:], in_=scaled[:], func=AF.Exp)

            # h = 1 / (1 + exp(-z)) for the 8 sigmoid outputs
            hden = smalls.tile([P, 8], F32, tag="hden", name=f"hden{t}")
            nc.vector.tensor_scalar_add(hden[:], eall[:, 0:8], 1.0)
            hps = smalls.tile([P, 8], F32, tag="hps", name=f"hps{t}")
            nc.vector.reciprocal(out=hps[:], in_=hden[:])

            # li early: libf = sum_m h_pre_m * x_m (DVE, bf16), then its
            # xbar transpose fires while the rest of the chain runs.
            libf = lip.tile([P, C], BF16, tag="libf", name=f"libf{t}")
            nc.vector.tensor_scalar_mul(libf[:], xn[:, 0:C], hps[:, 0:1])
            for m in range(1, N):
                nc.vector.scalar_tensor_tensor(
                    out=libf[:], in0=xn[:, m * C:(m + 1) * C],
                    scalar=hps[:, m:m + 1], in1=libf[:],
                    op0=ALU.mult, op1=ALU.add,
                )
            liT = lip.tile([P, 16, P], BF16, tag="liT", name=f"liT{t}")
            nc.sync.dma_start_transpose(out=liT[:], in_=libf[:])

            expsT_p = ps_small.tile([NFACT, P], F32, tag="pssmall", name=f"exT{t}")
            nc.tensor.transpose(expsT_p[:], eall[:, 8:32], idf32_s[:])
            expsT_s = smalls.tile([NFACT, P], F32, tag="expsT_s", name=f"exs{t}")
            nc.scalar.activation(out=expsT_s[:], in_=expsT_p[:], func=AF.Copy)

            haug_p = ps_small.tile([P, 17], F32, tag="pssmall", name=f"hgp{t}")
            nc.tensor.matmul(
                haug_p[:], expsT_s[:], perm_s[:], start=True, stop=True
            )
            # normalize straight out of PSUM: drops an ACT copy plus two
            # cross-engine hops from the per-beat critical chain
            dinv = smalls.tile([P, 1], F32, tag="dinv", name=f"dinv{t}")
            nc.vector.reciprocal(out=dinv[:], in_=haug_p[:, 16:17])

            # coeffs cols 0:16 = normalized H (col 4m+n = H[n,m]);
            # 16:20 = 2*h_post
            coeffs = smalls.tile([P, 20], F32, tag="coeffs", name=f"co{t}")
            nc.vector.tensor_scalar_mul(
                coeffs[:, 0:16], haug_p[:, 0:16], dinv[:]
            )
            nc.vector.tensor_scalar_mul(coeffs[:, 16:20], hps[:, 4:8], 2.0)

            # diags: j=4m+n -> H[n,m], built n-major so the first mixing
            # matmuls' stationaries land after 4 DVE ops instead of 13
            diags = diagp.tile([P, 16, P], BF16, tag="diags", name=f"dg{t}")
            for n in range(N):
                for src_ in range(N):
                    j = 4 * src_ + n
                    nc.vector.tensor_scalar_mul(
                        diags[:, j, :], idbf_s[:], coeffs[:, j:j + 1]
                    )

            st["diags"] = diags
            st["coeffs"] = coeffs
            st["liT"] = liT
            return st

        def stage_b(st):
            """diff = liT.T @ (W.T - I) + b, mixing, store."""
            xn = st["xn"]
            diags = st["diags"]
            rows = st["rows"]
            coeffs = st["coeffs"]
            liT = st["liT"]
            t = st["t"]

            diffbf = lip.tile([P, C], BF16, tag="diffbf", name=f"diffbf{t}")
            for q in range(4):
                cs = slice(q * 512, (q + 1) * 512)
                diff_p = ps_diff.tile([P, 512], F32, tag="diff")
                for k in range(16):
                    nc.tensor.matmul(
                        diff_p[:], liT[:, k, :], wp_s[:, k, cs],
                        start=(k == 0), stop=(k == 15),
                    )
                # diffbf = diff + b_layer (broadcast), cast to bf16
                nc.vector.scalar_tensor_tensor(
                    out=diffbf[:, cs], in0=diff_p[:], scalar=1.0,
                    in1=blb[:, cs], op0=ALU.bypass, op1=ALU.add,
                )

            # ---- mixing: out_n = sum_m diag(H[n,m]) @ x_m + h_post2_n*diff
            for n in range(N):
                outsb = xgp.tile([P, C], F32, tag="outsb", name=f"ou{t}_{n}")
                for cc in range(4):
                    cs = slice(cc * 512, (cc + 1) * 512)
                    mix_p = ps_mix.tile([P, 512], F32, tag="mix",
                                        name=f"mx{t}_{n}_{cc}")
                    for src_ in range(N):
                        nc.tensor.matmul(
                            mix_p[:], diags[:, 4 * src_ + n, :],
                            xn[:, src_ * C + cc * 512: src_ * C + (cc + 1) * 512],
                            start=(src_ == 0), stop=(src_ == 3),
                        )
                    nc.vector.scalar_tensor_tensor(
                        out=outsb[:, cs], in0=diffbf[:, cs],
                        scalar=coeffs[:, 16 + n:17 + n], in1=mix_p[:],
                        op0=ALU.mult, op1=ALU.add,
                    )
                nc.sync.dma_start(
                    out=out_d[rows, n * C:(n + 1) * C], in_=outsb[:]
                )

        # ---- software-pipelined emission ----
        # Per iteration: loads(t+1) first (DMA queues fill early), then the
        # heavy PE work of tile t (diff+mix), then tile t+1's coefficient
        # chain. Keeps ready work at each engine FIFO's head: tile t+1's
        # proj/diags (gated on DMA) never sit ahead of tile t's diff/mix.
        first = True
        pending = None  # tile with coeffs done, stage_b outstanding
        for rep in range(reps):
            for t in range(nt):
                ld = stage_load(t)
                if first:
                    # defer the big weight load until after tile 0's x DMAs
                    nc.sync.dma_start(out=wp_s[:], in_=wp_d[:])
                    first = False
                if pending is not None:
                    stage_b(pending)
                pending = stage_coeff(ld)
        stage_b(pending)


def prep_params(inputs):
    """Host-side parameter preprocessing shared by all cores."""
    bf = ml_dtypes.bfloat16
    W_all = np.asarray(inputs["W_all"], np.float32)
    W_layer = np.asarray(inputs["W_layer"], np.float32)
    b_all = np.asarray(inputs["b_all"], np.float32)
    b_layer = np.asarray(inputs["b_layer"], np.float32)
    perm_mat = np.asarray(inputs["perm_mat"], np.float32)
    a_pre = float(np.asarray(inputs["alpha_pre"]).reshape(-1)[0])
    a_post = float(np.asarray(inputs["alpha_post"]).reshape(-1)[0])
    a_res = float(np.asarray(inputs["alpha_res"]).reshape(-1)[0])

    wallt = np.ascontiguousarray(
        W_all.T.astype(bf).reshape(64, P, 32).transpose(1, 0, 2)
    )
    wp = (np.ascontiguousarray(W_layer.T) - np.eye(C, dtype=np.float32))
    wp = np.ascontiguousarray(wp.astype(bf).reshape(16, P, C).transpose(1, 0, 2))
    blayer = b_layer.astype(bf).reshape(1, C)
    # perm_aug columns in m-major order: col 4m+n = perm_mat[:, n*4+m]; col 16 = 1
    perm_aug = np.zeros((NFACT, 17), np.float32)
    perm_aug[:, :16] = perm_mat.reshape(NFACT, N, N).transpose(0, 2, 1).reshape(
        NFACT, 16
    )
    perm_aug[:, 16] = 1.0
    # cols 0:8 negated: eall = exp(-(alpha*p + b)) there, for sigmoid via 1/(1+u)
    alphab = np.zeros((2, 32), np.float32)
    alphab[0, 0:4] = -a_pre
    alphab[0, 4:8] = -a_post
    alphab[0, 8:32] = a_res
    alphab[1, 0:4] = -b_all[0:4]
    alphab[1, 4:8] = -b_all[4:8]
    alphab[1, 8:32] = b_all[8:32]
    idbf = np.eye(P, dtype=np.float32).astype(bf)
    idf32 = np.eye(P, dtype=np.float32)
    return {
        "wallt": wallt, "wp": wp, "blayer": blayer,
        "permaug": perm_aug, "alphab": alphab, "idbf": idbf, "idf32": idf32,
    }


_PROGRAM_CACHE = {}


def get_program(t_core):
    if t_core not in _PROGRAM_CACHE:
        _PROGRAM_CACHE[t_core] = build_program(t_core)
    return _PROGRAM_CACHE[t_core]


def run(inputs, trace=False):
    x = np.asarray(inputs["x_streams"], np.float32).reshape(T, NCF)
    params = prep_params(inputs)
    t_core = T // NCORES
    nc = get_program(t_core)
    in_maps = []
    for c in range(NCORES):
        m = dict(params)
        m["x"] = np.ascontiguousarray(x[c * t_core:(c + 1) * t_core])
        in_maps.append(m)
    res = bass_utils.run_bass_kernel_spmd(
        nc, in_maps, core_ids=list(range(NCORES)), trace=trace
    )
    out = np.concatenate([r["out"] for r in res.results], axis=0)
    return out.reshape(T, N, C).astype(np.float32), res


def kernel(**inputs) -> np.ndarray:
    out, _ = run(inputs)
    return out


def bench_reps(inputs, reps=5, calls=7):
    """Single-core timing: diff a reps-unrolled program against reps=1.

    Inputs are device-resident; each call is one NEFF execution, so the
    difference isolates (reps-1) kernel-body repetitions.
    """
    import time as _time

    import jax

    from concourse import bass2jax
    from concourse import mybir as _mb

    x = np.asarray(inputs["x_streams"], np.float32).reshape(T, NCF)
    params = prep_params(inputs)
    t_core = T // NCORES
    bass2jax.install_neuronx_cc_hook()

    results = {}
    for r in (1, reps):
        nc = build_program(t_core, reps=r, num_devices=1)
        partition_name = (
            nc.partition_id_tensor.name if nc.partition_id_tensor else None
        )
        in_names, out_names, out_avals, zero_outs = [], [], [], []
        for alloc in nc.m.functions[0].allocations:
            if not isinstance(alloc, _mb.MemoryLocationSet):
                continue
            name = alloc.memorylocations[0].name
            if alloc.kind == "ExternalInput":
                if name != partition_name:
                    in_names.append(name)
            elif alloc.kind == "ExternalOutput":
                out_names.append(name)
                shape = tuple(alloc.tensor_shape)
                dtype = _mb.dt.np(alloc.dtype)
                out_avals.append(jax.core.ShapedArray(shape, dtype))
                zero_outs.append(np.zeros(shape, dtype))
        bind_names = list(in_names) + list(out_names)
        if partition_name is not None:
            bind_names.append(partition_name)

        def _body(*flat, _nc=nc, _bind=tuple(bind_names),
                  _outn=tuple(out_names), _avals=tuple(out_avals),
                  _pn=partition_name):
            operands = list(flat)
            if _pn is not None:
                operands.append(bass2jax.partition_id_tensor())
            return tuple(bass2jax._bass_exec_p.bind(
                *operands, out_avals=_avals, in_names=_bind, out_names=_outn,
                lowering_input_output_aliases=(),
                sim_require_finite=True, sim_require_nnan=True, nc=_nc,
            ))

        m = dict(params)
        m["x"] = np.ascontiguousarray(x[:t_core])
        dev = jax.devices()[0]
        args = [jax.device_put(np.asarray(m[n]), dev) for n in in_names]
        args += [jax.device_put(z, dev) for z in zero_outs]
        fn = jax.jit(_body)
        outs = fn(*args)
        jax.block_until_ready(outs)
        best = None
        for _ in range(calls):
            t0 = _time.perf_counter()
            outs = fn(*args)
            jax.block_until_ready(outs)
            dt = _time.perf_counter() - t0
            best = dt if best is None else min(best, dt)
        results[r] = best
        print(f"  reps={r}: best call {best*1e3:.3f} ms")
    ns = (results[reps] - results[1]) / (reps - 1) * 1e9
    return ns


def bench(inputs, iters=8):
    """Time `iters` chained kernel executions on HW inside one jit.

    Outputs are fed back as the (normally zero-initialized) output buffers of
    the next iteration; the kernel overwrites every output element, so values
    stay correct and the data dependency serializes executions on-device.
    Returns (ns_per_iter, out_of_last_iter).
    """
    import jax
    from jax.sharding import Mesh, PartitionSpec
    from jax.experimental.shard_map import shard_map
    import time as _time
    from concourse import bass2jax, mybir as _mb

    x = np.asarray(inputs["x_streams"], np.float32).reshape(T, NCF)
    params = prep_params(inputs)
    t_core = T // NCORES
    nc = get_program(t_core)
    bass2jax.install_neuronx_cc_hook()

    partition_name = (
        nc.partition_id_tensor.name if nc.partition_id_tensor else None
    )
    in_names, out_names, out_avals, zero_outs = [], [], [], []
    for alloc in nc.m.functions[0].allocations:
        if not isinstance(alloc, _mb.MemoryLocationSet):
            continue
        name = alloc.memorylocations[0].name
        if alloc.kind == "ExternalInput":
            if name != partition_name:
                in_names.append(name)
        elif alloc.kind == "ExternalOutput":
            out_names.append(name)
            shape = tuple(alloc.tensor_shape)
            dtype = _mb.dt.np(alloc.dtype)
            out_avals.append(jax.core.ShapedArray(shape, dtype))
            zero_outs.append(np.zeros(shape, dtype))
    n_params = len(in_names)

    bind_names = list(in_names) + list(out_names)
    if partition_name is not None:
        bind_names.append(partition_name)

    def body_once(args, outs):
        operands = list(args) + list(outs)
        if partition_name is not None:
            operands.append(bass2jax.partition_id_tensor())
        res = bass2jax._bass_exec_p.bind(
            *operands,
            out_avals=tuple(out_avals),
            in_names=tuple(bind_names),
            out_names=tuple(out_names),
            lowering_input_output_aliases=(),
            sim_require_finite=True,
            sim_require_nnan=True,
            nc=nc,
        )
        return tuple(res)

    def chain(k):
        def _body(*flat):
            args = flat[:n_params]
            outs = flat[n_params:]
            for _ in range(k):
                outs = body_once(args, outs)
            return outs
        return _body

    devices = jax.devices()[:NCORES]
    mesh = Mesh(np.asarray(devices), ("core",))
    in_specs = (PartitionSpec("core"),) * (n_params + len(out_names))
    out_specs = (PartitionSpec("core"),) * len(out_names)

    per_core = []
    for c in range(NCORES):
        m = dict(params)
        m["x"] = np.ascontiguousarray(x[c * t_core:(c + 1) * t_core])
        per_core.append([np.asarray(m[n]) for n in in_names])
    concat_in = [
        np.concatenate([per_core[c][i] for c in range(NCORES)], axis=0)
        for i in range(n_params)
    ]
    concat_zeros = [
        np.zeros((NCORES * z.shape[0], *z.shape[1:]), z.dtype) for z in zero_outs
    ]

    times = {}
    for k in (1, 1 + iters):
        fn = jax.jit(
            shard_map(chain(k), mesh=mesh, in_specs=in_specs,
                      out_specs=out_specs, check_rep=False)
        )
        out_arrs = fn(*concat_in, *concat_zeros)  # compile+warm
        jax.block_until_ready(out_arrs)
        reps = 3
        best = None
        for _ in range(reps):
            t0 = _time.perf_counter()
            out_arrs = fn(*concat_in, *concat_zeros)
            jax.block_until_ready(out_arrs)
            dt = _time.perf_counter() - t0
            best = dt if best is None else min(best, dt)
        times[k] = best
    ns = (times[1 + iters] - times[1]) / iters * 1e9
    out = np.asarray(out_arrs[0]).reshape(NCORES, t_core, NCF).reshape(T, N, C)
    return ns, out

